# revision 2
# baseline (speedup 1.0000x reference)
"""Trainium2 Bass kernel for CoreProcessor (soft memory-slot routing), v2.

Computation (per token t):
    q = x Wq^T + bq
    a = softmax((q keys^T) / sqrt(d))
    out = sum_m a[m] * (ops[m] @ x)

Sharding: data-parallel over the 16384 tokens across 8 cores (2048 each);
keys/ops/Wq/bq replicated.

v2 design (vs v1): all static layout work moved to the HOST:
  - xT bf16 [d, tok] (host-transposed), opsT bf16 [e, (m,d)] (host),
    K2 = (keys Wq)^T / sqrt(d) bf16 [d, m], bl8 = bias row [1, 8*m] bf16.
    No device prologue, no per-tile transposes, no ACT xT copies.
  - Heads batched 8 tiles per group: one PSUM bank of logits (8 matmuls +
    1 bias matmul), one 512-wide exp, one strided Z-reduce, reciprocal,
    and 1/Z folded into p up front (p8n) so no output scaling remains.
  - Mains: 16 rhs chunks of 512 per tile; PSUM organized as two 2-bank
    double positions + one single position (+1 logits bank + 2 pe_acc).
  - Drain routes per tile (grid-tuned):
      Dd: DVE 1024-wide tensor_tensor (y * p bcast) -> z bf16; PE identity
          matmuls reduce z into pe_acc.
      Ad: ACT 1024-wide copy -> y bf16 SBUF; Pool scalar_tensor_tensor per
          slot fuses scale+accumulate into pool_acc (f32) - no PE idents.
      S:  ACT per-slot scaled copies (512 chunk) -> z bf16; PE idents.
  - Final per tile: one DVE scalar_tensor_tensor joins pe_acc (PSUM) +
    pool_acc -> out tile; DMA out. Idents of tile i are interleaved
    between tile i+1's mains so the in-order PE never stalls.
"""

import sys

import numpy as np

sys.path.insert(0, "/opt/trn_rl_repo")

import concourse.bass as bass  # noqa: E402
import concourse.tile as tile  # noqa: E402
from concourse import bacc, mybir  # noqa: E402
from concourse.bass_utils import run_bass_kernel_spmd  # noqa: E402
from concourse.masks import make_identity  # noqa: E402

F32 = mybir.dt.float32
BF16 = mybir.dt.bfloat16

N_CORES = 8
B, S, D, M = 4, 4096, 128, 64
NTOK_TOTAL = B * S            # 16384
NTOK = NTOK_TOTAL // N_CORES  # 2048 tokens per core
NT = NTOK // 128              # 16 token tiles per core
NCHUNK = (M * D) // 512       # 16 rhs chunks of 512 (4 slots each)
GT = 8                        # tiles per head group
SCALE = 1.0 / float(np.sqrt(np.float32(D)))

_CACHE = {}


def _build(plan=None, ids_per_main=None, merges=1, zbufs=2, ybufs=2, obufs=3, last_self=False):
    """plan: per-parity tile route lists. Each tile's plan is a list of
    ('Dd'|'Ad', dbl_idx) doubles and ('S', sgl_idx) singles covering 16
    chunks (each Dd/Ad = 2 chunks, S = 1)."""
    from contextlib import ExitStack

    if plan is None:
        # Per-tile route tokens: D = DVE-bcast double (2 chunks),
        # A = ACT-copy + Pool-STT double, V = DVE-bcast single,
        # S = ACT slot-scaled single. Must cover 16 chunks.
        plan = {
            0: ["A", "A", "D", "S", "A", "D", "S", "D", "D"],
            1: ["A", "A", "D", "S", "A", "D", "S", "D", "D"],
        }
    # idents of the previous tile emitted after the k-th main of this tile
    if ids_per_main is None:
        ids_per_main = [3] * 12 + [1] * 4
    if isinstance(ids_per_main, int):
        ids_sched = [ids_per_main] * 24
    else:
        ids_sched = list(ids_per_main) + [0] * 24

    nc = bacc.Bacc("TRN2", target_bir_lowering=False, debug=False)

    xT_d = nc.dram_tensor("xT", [D, NTOK], BF16, kind="ExternalInput")
    opsT_d = nc.dram_tensor("opsT", [D, M * D], BF16, kind="ExternalInput")
    k2_d = nc.dram_tensor("k2", [D, M], BF16, kind="ExternalInput")
    bl8_d = nc.dram_tensor("bl8", [1, GT * M], BF16, kind="ExternalInput")
    out_d = nc.dram_tensor("out", [NTOK, D], F32, kind="ExternalOutput")

    with tile.TileContext(nc) as tc, ExitStack() as ctx:
        consts = ctx.enter_context(tc.tile_pool(name="consts", bufs=1))
        p_pool = ctx.enter_context(tc.tile_pool(name="p", bufs=2))
        z_pool = ctx.enter_context(tc.tile_pool(name="z", bufs=zbufs))
        y_pool = ctx.enter_context(tc.tile_pool(name="ybf", bufs=ybufs))
        acc_pool = ctx.enter_context(tc.tile_pool(name="acc", bufs=2))
        out_pool = ctx.enter_context(tc.tile_pool(name="outp", bufs=obufs))
        small = ctx.enter_context(tc.tile_pool(name="small", bufs=4))
        pa_ps = ctx.enter_context(
            tc.tile_pool(name="paps", bufs=2, space=bass.MemorySpace.PSUM)
        )
        yd_ps = ctx.enter_context(
            tc.tile_pool(name="ydps", bufs=2, space=bass.MemorySpace.PSUM)
        )
        # singles + the head's logits bank share one 2-slot ring (same tag)
        ys_ps = ctx.enter_context(
            tc.tile_pool(name="ysps", bufs=2, space=bass.MemorySpace.PSUM)
        )

        # ---- constants / inputs ----
        ident = consts.tile([128, 128], F32)
        make_identity(nc, ident)
        ident_bf = consts.tile([128, 128], BF16)
        nc.vector.tensor_copy(ident_bf, ident)
        ones_bf = consts.tile([1, 128], BF16)
        nc.vector.memset(ones_bf, 1.0)

        k2_sb = consts.tile([D, M], BF16)
        nc.sync.dma_start(k2_sb, k2_d[:])
        bl8_sb = consts.tile([1, GT * M], BF16)
        xT_sb = consts.tile([D, NTOK], BF16)
        # first half of xT (head group 0 + early tiles) lands first
        nc.sync.dma_start(xT_sb[:, :NTOK // 2], xT_d[:, :NTOK // 2])
        nc.sync.dma_start(bl8_sb, bl8_d[:])
        # opsT quarters as separate tiles for tile-granular dep tracking;
        # q2/q3 go via the (idle) gpsimd SWDGE queue in parallel with the
        # SP HWDGE queue
        MQ = M // 4
        opsT_q = [
            consts.tile([D, MQ * D], BF16, name=f"opsT_q{q}") for q in range(4)
        ]
        nc.sync.dma_start(opsT_q[0], opsT_d[:, 0:MQ * D])
        nc.sync.dma_start(xT_sb[:, NTOK // 2:], xT_d[:, NTOK // 2:])
        nc.sync.dma_start(opsT_q[1], opsT_d[:, MQ * D:2 * MQ * D])
        nc.sync.dma_start(opsT_q[2], opsT_d[:, 2 * MQ * D:3 * MQ * D])
        nc.sync.dma_start(opsT_q[3], opsT_d[:, 3 * MQ * D:4 * MQ * D])

        def ops_slice(lo_slot, n_slots):
            """opsT columns for slots [lo_slot, lo_slot+n_slots) (must stay
            within one quarter)."""
            q = lo_slot // MQ
            base = (lo_slot - q * MQ) * D
            return opsT_q[q][:, base:base + n_slots * D]

        # ---- heads: one group of 8 tiles ----
        heads = {}

        def emit_head(g):
            lp8 = ys_ps.tile([128, GT * M], F32, tag="ys", name="lp8")
            for k in range(GT):
                t0 = (g * GT + k) * 128
                nc.tensor.matmul(
                    lp8[:, k * M:(k + 1) * M],
                    xT_sb[:, t0:t0 + 128], k2_sb,
                    start=(k == 0), stop=False, skip_group_check=True,
                )
            nc.tensor.matmul(lp8, ones_bf, bl8_sb, start=False, stop=True,
                             skip_group_check=True)
            p8 = p_pool.tile([128, GT * M], F32, tag="p8")
            nc.scalar.activation(
                p8, lp8, mybir.ActivationFunctionType.Exp, bias=0.0, scale=1.0,
            )
            zs8 = small.tile([128, GT], F32, tag="zs")
            nc.vector.tensor_reduce(
                zs8, p8[:].rearrange("t (k m) -> t k m", m=M),
                axis=mybir.AxisListType.X, op=mybir.AluOpType.add,
            )
            rz8 = small.tile([128, GT], F32, tag="rz")
            nc.vector.reciprocal(rz8, zs8)
            p8n = p_pool.tile([128, GT * M], F32, tag="p8n")
            nc.vector.tensor_tensor(
                p8n[:].rearrange("t (k m) -> t k m", m=M),
                p8[:].rearrange("t (k m) -> t k m", m=M),
                rz8[:].unsqueeze(2).broadcast_to([128, GT, M]),
                op=mybir.AluOpType.mult,
            )
            for k in range(GT):
                heads[g * GT + k] = (p8n, k)

        def emit_idents(pend, lo, hi):
            z_tiles, pe_acc = pend[0], pend[1]
            n = len(z_tiles)
            for k in range(min(lo, n), min(hi, n)):
                nc.tensor.matmul(
                    pe_acc, ident_bf, z_tiles[k],
                    start=(k == 0), stop=(k == n - 1),
                    skip_group_check=True,
                )

        def emit_join(pend):
            """out_tile = pe_acc + pool_acc, then DMA."""
            _, pe_acc, pool_acc, ti = pend
            out_t = out_pool.tile([128, 128], F32)
            if pool_acc is not None:
                nc.vector.scalar_tensor_tensor(
                    out_t, pe_acc, 1.0, pool_acc,
                    op0=mybir.AluOpType.mult, op1=mybir.AluOpType.add,
                )
            else:
                nc.vector.tensor_copy(out_t, pe_acc)
            nc.sync.dma_start(out_d[ti * 128:(ti + 1) * 128, :], out_t)

        def emit_body(i, pend, self_drain=False):
            p8n, kk = heads.pop(i)
            pbase = kk * M
            xT_t = xT_sb[:, i * 128:(i + 1) * 128]
            pe_acc = pa_ps.tile([128, 128], F32, tag="pacc")
            pool_acc = None
            z_tiles = []
            z_dbls = []
            routes = plan[i % 2]
            slot = 0
            nid = 0  # idents of prev emitted so far
            own_nid = 0
            ndrain = 0

            nmain = 0

            def fill_idents():
                nonlocal nid, nmain
                k = ids_sched[nmain]
                nmain += 1
                if pend:
                    emit_idents(pend, nid, nid + k)
                    nid += k

            entry_counts = []
            for r in routes:
                entry_counts.append(len(z_tiles))
                if r == "D":
                    yd = yd_ps.tile([128, 1024], F32, tag="ydd")
                    nc.tensor.matmul(yd[:, :512], xT_t, ops_slice(slot, 4),
                                     start=True, stop=True,
                                     skip_group_check=True)
                    fill_idents()
                    nc.tensor.matmul(yd[:, 512:], xT_t, ops_slice(slot + 4, 4),
                                     start=True, stop=True,
                                     skip_group_check=True)
                    fill_idents()
                    zd = z_pool.tile([128, 1024], BF16, tag=f"zd{slot}",
                                     name=f"zd{slot}")
                    nc.vector.tensor_tensor(
                        zd[:].rearrange("t (m e) -> t m e", e=128),
                        yd[:].rearrange("t (m e) -> t m e", e=128),
                        p8n[:, pbase + slot:pbase + slot + 8]
                            .unsqueeze(2).broadcast_to([128, 8, 128]),
                        op=mybir.AluOpType.mult,
                    )
                    if self_drain:
                        for j in range(8):
                            z_tiles.append(zd[:, j * 128:(j + 1) * 128])
                    else:
                        z_dbls.append((0, zd))
                    slot += 8
                elif r == "A":
                    yd = yd_ps.tile([128, 1024], F32, tag="ydd")
                    nc.tensor.matmul(yd[:, :512], xT_t, ops_slice(slot, 4),
                                     start=True, stop=True,
                                     skip_group_check=True)
                    fill_idents()
                    nc.tensor.matmul(yd[:, 512:], xT_t, ops_slice(slot + 4, 4),
                                     start=True, stop=True,
                                     skip_group_check=True)
                    fill_idents()
                    yb = y_pool.tile([128, 1024], BF16, tag=f"yb{slot}",
                                     name=f"yb{slot}")
                    nc.scalar.copy(yb, yd)
                    za = z_pool.tile([128, 1024], BF16, tag=f"za{slot}",
                                     name=f"za{slot}")
                    nc.gpsimd.tensor_tensor(
                        za[:].rearrange("t (m e) -> t m e", e=128),
                        yb[:].rearrange("t (m e) -> t m e", e=128),
                        p8n[:, pbase + slot:pbase + slot + 8]
                            .unsqueeze(2).broadcast_to([128, 8, 128]),
                        op=mybir.AluOpType.mult,
                    )
                    if self_drain:
                        for j in range(8):
                            z_tiles.append(za[:, j * 128:(j + 1) * 128])
                    else:
                        z_dbls.append((1, za))
                    slot += 8
                elif r == "V":
                    ys = ys_ps.tile([128, 512], F32, tag="ys", name="ys")
                    nc.tensor.matmul(ys, xT_t, ops_slice(slot, 4),
                                     start=True, stop=True,
                                     skip_group_check=True)
                    fill_idents()
                    zv = z_pool.tile([128, 512], BF16, tag=f"zv{slot}",
                                     name=f"zv{slot}")
                    nc.vector.tensor_tensor(
                        zv[:].rearrange("t (m e) -> t m e", e=128),
                        ys[:].rearrange("t (m e) -> t m e", e=128),
                        p8n[:, pbase + slot:pbase + slot + 4]
                            .unsqueeze(2).broadcast_to([128, 4, 128]),
                        op=mybir.AluOpType.mult,
                    )
                    for j in range(4):
                        z_tiles.append(zv[:, j * 128:(j + 1) * 128])
                    slot += 4
                else:  # "S"
                    ys = ys_ps.tile([128, 512], F32, tag="ys", name="ys")
                    nc.tensor.matmul(ys, xT_t, ops_slice(slot, 4),
                                     start=True, stop=True,
                                     skip_group_check=True)
                    fill_idents()
                    for j in range(4):
                        zs = z_pool.tile([128, 128], BF16, tag=f"zs{slot + j}",
                                         name=f"zs{slot + j}")
                        nc.scalar.mul(
                            zs, ys[:, j * 128:(j + 1) * 128],
                            p8n[:, pbase + slot + j:pbase + slot + j + 1],
                        )
                        z_tiles.append(zs)
                    slot += 4
                ndrain += 2 if r in ("D", "A") else 1
                lag_n = (entry_counts[-3] if len(entry_counts) >= 3
                         else 0)
                if self_drain and lag_n > own_nid:
                    for k in range(own_nid, lag_n):
                        nc.tensor.matmul(
                            pe_acc, ident_bf, z_tiles[k],
                            start=(k == 0), stop=False,
                            skip_group_check=True,
                        )
                    own_nid = lag_n
            assert slot == M, f"plan covers {slot} slots"
            # sort by drain-engine speed: DVE z first, Pool-scaled z last
            z_dbls.sort(key=lambda kz: kz[0])
            # pair-merge the FIRST z doubles on DVE (bf16 2x adds) to cut
            # their ident count
            for k in range(merges):
                if len(z_dbls) < 2:
                    break
                (ka, za), (kb, zb) = z_dbls.pop(0), z_dbls.pop(0)
                zm = z_pool.tile([128, 1024], BF16, tag=f"zm{k}",
                                 name=f"zm{k}")
                nc.vector.tensor_add(zm, za, zb)
                z_dbls.append((2, zm))
            for _, zd in z_dbls:
                for j in range(8):
                    z_tiles.append(zd[:, j * 128:(j + 1) * 128])
            if pend:
                emit_idents(pend, nid, len(pend[0]))  # flush stragglers
                emit_join(pend)
            if self_drain:
                n = len(z_tiles)
                for k in range(own_nid, n):
                    nc.tensor.matmul(
                        pe_acc, ident_bf, z_tiles[k],
                        start=(k == 0), stop=(k == n - 1),
                        skip_group_check=True,
                    )
                if own_nid >= n:
                    # all emitted with stop=False; emit a closing no-op
                    # accumulation of the last z to terminate the group
                    nc.tensor.matmul(
                        pe_acc, ident_bf, z_tiles[n - 1],
                        start=False, stop=True, skip_group_check=True,
                    )
                    raise AssertionError("unreachable: lag ensures tail")
                emit_join((None, pe_acc, pool_acc, i))
                return None
            return (z_tiles, pe_acc, pool_acc, i)

        emit_head(0)
        pend = None
        for i in range(NT):
            if i % GT == GT - 3 and i // GT + 1 < NT // GT:
                emit_head(i // GT + 1)
            pend = emit_body(i, pend, self_drain=(i == NT - 1 and last_self))
        if pend is not None:
            emit_idents(pend, 0, len(pend[0]))
            emit_join(pend)

    nc.compile()
    return nc


def _get_nc(**kw):
    key = repr(sorted(kw.items()))
    if key not in _CACHE:
        _CACHE[key] = _build(**kw)
    return _CACHE[key]


def _host_prep(inputs):
    import ml_dtypes
    x = np.asarray(inputs["input_tensor"], np.float32).reshape(NTOK_TOTAL, D)
    keys = np.asarray(inputs["memory_keys"], np.float32)
    ops = np.asarray(inputs["memory_ops"], np.float32)
    wq = np.asarray(inputs["Wq"], np.float32)
    bq = np.asarray(inputs["bq"], np.float32)

    bf16 = ml_dtypes.bfloat16
    # logits = x @ K2 + bl ; K2[d, m] = sum_e Wq[e, d] keys[m, e] / sqrt(D)
    k2 = ((keys @ wq).T * SCALE).astype(bf16)                  # [D, M]
    bl = ((keys @ bq) * SCALE).astype(np.float32)              # [M]
    bl8 = np.ascontiguousarray(np.tile(bl, GT)[None, :]).astype(bf16)
    opsT = np.ascontiguousarray(
        ops.transpose(2, 0, 1).reshape(D, M * D)).astype(bf16)  # [e, (m,d)]
    xT = [
        np.ascontiguousarray(x[c * NTOK:(c + 1) * NTOK].T).astype(bf16)
        for c in range(N_CORES)
    ]
    return xT, opsT, k2, bl8


def _run(inputs, trace=False, **build_kw):
    nc = _get_nc(**build_kw)
    xT, opsT, k2, bl8 = _host_prep(inputs)
    in_maps = [
        {"xT": xT[c], "opsT": opsT, "k2": k2, "bl8": bl8}
        for c in range(N_CORES)
    ]
    res = run_bass_kernel_spmd(
        nc, in_maps, core_ids=list(range(N_CORES)), trace=trace
    )
    out = np.concatenate(
        [res.results[c]["out"] for c in range(N_CORES)], axis=0)
    return out.reshape(B, S, D), res


def kernel(**inputs) -> np.ndarray:
    out, _ = _run(inputs, trace=False)
    return out


# revision 4
# speedup vs baseline: 1.0196x; 1.0196x over previous
"""Trainium2 Bass kernel for CoreProcessor (soft memory-slot routing).

Computation (per token t):
    q = x Wq^T + bq
    a = softmax((q keys^T) / sqrt(d))
    out = sum_m a[m] * (ops[m] @ x)

Sharding: data-parallel over the 16384 tokens across 8 cores (2048 each);
keys/ops/Wq/bq replicated.

All static layout work is done on the HOST:
  - xT bf16 [d, 64+tok] (host-transposed, with K2 = (keys Wq)^T/sqrt(d)
    packed as a 64-col header so one DMA starts the head chain),
    opsT bf16 [e, (m,d)], bl8 bias row [1, 8m] bf16. No device prologue,
    no per-tile transposes, no ACT xT copies.
  - Heads batched 8 tiles per group: one PSUM bank of logits (8 matmuls +
    1 bias matmul), one 512-wide exp, strided Z-reduce, reciprocal, and
    1/Z folded into p up front (p8n) so no output scaling remains.
  - Mains: 16 rhs chunks of 512 per tile. PSUM: two 2-bank double
    positions + one single-bank ring (shared with the logits bank via a
    common tile tag) + two rotating pe_acc banks = 8 banks.
  - Drain routes per tile (plan "AADSADSDD" = A:3 D:4 S:2 per tile,
    grid-tuned against the cost model; Pool has no PSUM port and rejects
    TensorScalarPtr, so A goes through ACT+Pool TT):
      D: DVE 1024-wide tensor_tensor (y * p bcast) -> z bf16.
      A: ACT 1024-wide copy -> bf16 SBUF; Pool TT bcast-multiply scales.
      S: ACT per-slot scaled copies (512 chunk) -> z bf16.
    One DVE pair-merge (1024-wide bf16 add, 2x mode) halves two D-chunks'
    ident count. The m-reduction is PE identity-matmuls into pe_acc,
    interleaved between the NEXT tile's mains (z is consumed
    drain-speed-sorted: singles, DVE z, Pool z, merged z) so the in-order
    PE never stalls; the per-tile join (pe_acc -> out, DVE) is emitted
    only after every ident, then DMA out.
"""

import sys

import numpy as np

sys.path.insert(0, "/opt/trn_rl_repo")

import concourse.bass as bass  # noqa: E402
import concourse.tile as tile  # noqa: E402
from concourse import bacc, mybir  # noqa: E402
from concourse.bass_utils import run_bass_kernel_spmd  # noqa: E402
from concourse.masks import make_identity  # noqa: E402

F32 = mybir.dt.float32
BF16 = mybir.dt.bfloat16

N_CORES = 8
B, S, D, M = 4, 4096, 128, 64
NTOK_TOTAL = B * S            # 16384
NTOK = NTOK_TOTAL // N_CORES  # 2048 tokens per core
NT = NTOK // 128              # 16 token tiles per core
NCHUNK = (M * D) // 512       # 16 rhs chunks of 512 (4 slots each)
GT = 8                        # tiles per head group
SCALE = 1.0 / float(np.sqrt(np.float32(D)))

_CACHE = {}


def _build(plan=None, ids_per_main=None, merges=1, zbufs=2, ybufs=2, obufs=3, last_self=False, defer_merges=True, act_join=False):
    """plan: per-parity tile route lists. Each tile's plan is a list of
    ('Dd'|'Ad', dbl_idx) doubles and ('S', sgl_idx) singles covering 16
    chunks (each Dd/Ad = 2 chunks, S = 1)."""
    from contextlib import ExitStack

    if plan is None:
        # Per-tile route tokens: D = DVE-bcast double (2 chunks),
        # A = ACT-copy + Pool-STT double, V = DVE-bcast single,
        # S = ACT slot-scaled single. Must cover 16 chunks.
        plan = {
            0: ["A", "A", "D", "S", "A", "D", "S", "D", "D"],
            1: ["A", "A", "D", "S", "A", "D", "S", "D", "D"],
        }
    # idents of the previous tile emitted after the k-th main of this tile
    if ids_per_main is None:
        ids_per_main = [5] * 6 + [2] * 6 + [1] * 4
    if isinstance(ids_per_main, int):
        ids_sched = [ids_per_main] * 24
    else:
        ids_sched = list(ids_per_main) + [0] * 24

    nc = bacc.Bacc("TRN2", target_bir_lowering=False, debug=False)

    xT_d = nc.dram_tensor("xT", [D, M + NTOK], BF16, kind="ExternalInput")
    opsT_d = nc.dram_tensor("opsT", [D, M * D], BF16, kind="ExternalInput")
    k2_d = nc.dram_tensor("k2", [D, M], BF16, kind="ExternalInput")
    bl8_d = nc.dram_tensor("bl8", [1, GT * M], BF16, kind="ExternalInput")
    out_d = nc.dram_tensor("out", [NTOK, D], F32, kind="ExternalOutput")

    with tile.TileContext(nc) as tc, ExitStack() as ctx:
        consts = ctx.enter_context(tc.tile_pool(name="consts", bufs=1))
        p_pool = ctx.enter_context(tc.tile_pool(name="p", bufs=2))
        z_pool = ctx.enter_context(tc.tile_pool(name="z", bufs=zbufs))
        y_pool = ctx.enter_context(tc.tile_pool(name="ybf", bufs=ybufs))
        acc_pool = ctx.enter_context(tc.tile_pool(name="acc", bufs=2))
        out_pool = ctx.enter_context(tc.tile_pool(name="outp", bufs=obufs))
        small = ctx.enter_context(tc.tile_pool(name="small", bufs=4))
        pa_ps = ctx.enter_context(
            tc.tile_pool(name="paps", bufs=2, space=bass.MemorySpace.PSUM)
        )
        yd_ps = ctx.enter_context(
            tc.tile_pool(name="ydps", bufs=2, space=bass.MemorySpace.PSUM)
        )
        # singles + the head's logits bank share one 2-slot ring (same tag)
        ys_ps = ctx.enter_context(
            tc.tile_pool(name="ysps", bufs=2, space=bass.MemorySpace.PSUM)
        )

        # ---- constants / inputs ----
        ident = consts.tile([128, 128], F32)
        make_identity(nc, ident)
        ident_bf = consts.tile([128, 128], BF16)
        nc.vector.tensor_copy(ident_bf, ident)
        ones_bf = consts.tile([1, 128], BF16)
        nc.vector.memset(ones_bf, 1.0)

        # k2 rides as a 64-col header on xT: one DMA starts the head chain
        kx_sb = consts.tile([D, M + NTOK], BF16)
        k2_sb = kx_sb[:, :M]
        xT_sb = kx_sb[:, M:]
        nc.sync.dma_start(kx_sb[:, :M + NTOK // 2], xT_d[:, :M + NTOK // 2])
        bl8_sb = consts.tile([1, GT * M], BF16)
        nc.gpsimd.dma_start(bl8_sb, bl8_d[:])
        # opsT quarters as separate tiles for tile-granular dep tracking;
        # q2/q3 go via the (idle) gpsimd SWDGE queue in parallel with the
        # SP HWDGE queue
        MQ = M // 4
        opsT_q = [
            consts.tile([D, MQ * D], BF16, name=f"opsT_q{q}") for q in range(4)
        ]
        nc.sync.dma_start(opsT_q[0][:, :MQ * D // 2], opsT_d[:, 0:MQ * D // 2])
        nc.sync.dma_start(opsT_q[0][:, MQ * D // 2:], opsT_d[:, MQ * D // 2:MQ * D])
        nc.sync.dma_start(xT_sb[:, NTOK // 2:], xT_d[:, M + NTOK // 2:])
        nc.sync.dma_start(opsT_q[1], opsT_d[:, MQ * D:2 * MQ * D])
        nc.sync.dma_start(opsT_q[2], opsT_d[:, 2 * MQ * D:3 * MQ * D])
        nc.sync.dma_start(opsT_q[3], opsT_d[:, 3 * MQ * D:4 * MQ * D])

        def ops_slice(lo_slot, n_slots):
            """opsT columns for slots [lo_slot, lo_slot+n_slots) (must stay
            within one quarter)."""
            q = lo_slot // MQ
            base = (lo_slot - q * MQ) * D
            return opsT_q[q][:, base:base + n_slots * D]

        # ---- heads: one group of 8 tiles ----
        heads = {}

        def emit_head(g):
            lp8 = ys_ps.tile([128, GT * M], F32, tag="ys", name="lp8")
            for k in range(GT):
                t0 = (g * GT + k) * 128
                nc.tensor.matmul(
                    lp8[:, k * M:(k + 1) * M],
                    xT_sb[:, t0:t0 + 128], k2_sb,
                    start=(k == 0), stop=False, skip_group_check=True,
                )
            nc.tensor.matmul(lp8, ones_bf, bl8_sb, start=False, stop=True,
                             skip_group_check=True)
            p8 = p_pool.tile([128, GT * M], F32, tag="p8")
            nc.scalar.activation(
                p8, lp8, mybir.ActivationFunctionType.Exp, bias=0.0, scale=1.0,
            )
            zs8 = small.tile([128, GT], F32, tag="zs")
            nc.vector.tensor_reduce(
                zs8, p8[:].rearrange("t (k m) -> t k m", m=M),
                axis=mybir.AxisListType.X, op=mybir.AluOpType.add,
            )
            rz8 = small.tile([128, GT], F32, tag="rz")
            nc.vector.reciprocal(rz8, zs8)
            p8n = p_pool.tile([128, GT * M], F32, tag="p8n")
            nc.vector.tensor_tensor(
                p8n[:].rearrange("t (k m) -> t k m", m=M),
                p8[:].rearrange("t (k m) -> t k m", m=M),
                rz8[:].unsqueeze(2).broadcast_to([128, GT, M]),
                op=mybir.AluOpType.mult,
            )
            for k in range(GT):
                heads[g * GT + k] = (p8n, k)

        def emit_idents(pend, lo, hi):
            z_tiles, pe_acc = pend[0], pend[1]
            n = len(z_tiles)
            for k in range(min(lo, n), min(hi, n)):
                nc.tensor.matmul(
                    pe_acc, ident_bf, z_tiles[k],
                    start=(k == 0), stop=(k == n - 1),
                    skip_group_check=True,
                )

        def emit_join(pend):
            """out_tile = pe_acc (+ dve_acc), then DMA."""
            _, pe_acc, dve_acc, ti = pend[:4]
            out_t = out_pool.tile([128, 128], F32)
            if dve_acc is not None:
                nc.vector.scalar_tensor_tensor(
                    out_t, pe_acc, 1.0, dve_acc,
                    op0=mybir.AluOpType.mult, op1=mybir.AluOpType.add,
                )
            elif act_join:
                nc.scalar.copy(out_t, pe_acc)
            else:
                nc.vector.tensor_copy(out_t, pe_acc)
            nc.sync.dma_start(out_d[ti * 128:(ti + 1) * 128, :], out_t)

        def emit_body(i, pend, self_drain=False):
            p8n, kk = heads.pop(i)
            pbase = kk * M
            xT_t = xT_sb[:, i * 128:(i + 1) * 128]
            pe_acc = pa_ps.tile([128, 128], F32, tag="pacc")
            pool_acc = None
            dve_acc = None
            z_tiles = []
            z_dbls = []
            routes = plan.get(i, plan[i % 2]) if isinstance(plan, dict) \
                else plan[i % 2]
            slot = 0
            nid = 0  # idents of prev emitted so far
            own_nid = 0
            ndrain = 0

            nmain = 0

            def fill_idents():
                nonlocal nid, nmain
                k = ids_sched[nmain]
                nmain += 1
                if pend:
                    emit_idents(pend, nid, nid + k)
                    nid += k

            entry_counts = []
            n_entry = 0
            for r in routes:
                n_entry += 1
                if pend and n_entry == 2:
                    for zm, za, zb in pend[4]:
                        nc.vector.tensor_add(zm, za, zb)
                entry_counts.append(len(z_tiles))
                if r == "D":
                    yd = yd_ps.tile([128, 1024], F32, tag="ydd")
                    nc.tensor.matmul(yd[:, :512], xT_t, ops_slice(slot, 4),
                                     start=True, stop=True,
                                     skip_group_check=True)
                    fill_idents()
                    nc.tensor.matmul(yd[:, 512:], xT_t, ops_slice(slot + 4, 4),
                                     start=True, stop=True,
                                     skip_group_check=True)
                    fill_idents()
                    zd = z_pool.tile([128, 1024], BF16, tag=f"zd{slot}",
                                     name=f"zd{slot}")
                    nc.vector.tensor_tensor(
                        zd[:].rearrange("t (m e) -> t m e", e=128),
                        yd[:].rearrange("t (m e) -> t m e", e=128),
                        p8n[:, pbase + slot:pbase + slot + 8]
                            .unsqueeze(2).broadcast_to([128, 8, 128]),
                        op=mybir.AluOpType.mult,
                    )
                    if self_drain:
                        for j in range(8):
                            z_tiles.append(zd[:, j * 128:(j + 1) * 128])
                    else:
                        z_dbls.append((0, zd))
                    slot += 8
                elif r == "P":
                    # A-pair: two doubles copied into one 2048-wide bf16
                    # buffer, ONE Pool TT scales all 16 slots
                    yb2 = y_pool.tile([128, 2048], BF16, tag=f"yb2{slot}",
                                      name=f"yb2{slot}")
                    for h in range(2):
                        yd = yd_ps.tile([128, 1024], F32, tag="ydd")
                        nc.tensor.matmul(yd[:, :512], xT_t,
                                         ops_slice(slot + 8 * h, 4),
                                         start=True, stop=True,
                                         skip_group_check=True)
                        fill_idents()
                        nc.tensor.matmul(yd[:, 512:], xT_t,
                                         ops_slice(slot + 8 * h + 4, 4),
                                         start=True, stop=True,
                                         skip_group_check=True)
                        fill_idents()
                        nc.scalar.copy(yb2[:, h * 1024:(h + 1) * 1024], yd)
                    za2 = z_pool.tile([128, 2048], BF16, tag=f"za2{slot}",
                                      name=f"za2{slot}")
                    nc.gpsimd.tensor_tensor(
                        za2[:].rearrange("t (m e) -> t m e", e=128),
                        yb2[:].rearrange("t (m e) -> t m e", e=128),
                        p8n[:, pbase + slot:pbase + slot + 16]
                            .unsqueeze(2).broadcast_to([128, 16, 128]),
                        op=mybir.AluOpType.mult,
                    )
                    z_dbls.append((1, za2[:, :1024]))
                    z_dbls.append((1, za2[:, 1024:]))
                    slot += 16
                elif r == "A":
                    yd = yd_ps.tile([128, 1024], F32, tag="ydd")
                    nc.tensor.matmul(yd[:, :512], xT_t, ops_slice(slot, 4),
                                     start=True, stop=True,
                                     skip_group_check=True)
                    fill_idents()
                    nc.tensor.matmul(yd[:, 512:], xT_t, ops_slice(slot + 4, 4),
                                     start=True, stop=True,
                                     skip_group_check=True)
                    fill_idents()
                    yb = y_pool.tile([128, 1024], BF16, tag=f"yb{slot}",
                                     name=f"yb{slot}")
                    nc.scalar.copy(yb, yd)
                    za = z_pool.tile([128, 1024], BF16, tag=f"za{slot}",
                                     name=f"za{slot}")
                    nc.gpsimd.tensor_tensor(
                        za[:].rearrange("t (m e) -> t m e", e=128),
                        yb[:].rearrange("t (m e) -> t m e", e=128),
                        p8n[:, pbase + slot:pbase + slot + 8]
                            .unsqueeze(2).broadcast_to([128, 8, 128]),
                        op=mybir.AluOpType.mult,
                    )
                    if self_drain:
                        for j in range(8):
                            z_tiles.append(za[:, j * 128:(j + 1) * 128])
                    else:
                        z_dbls.append((1, za))
                    slot += 8
                elif r == "V":
                    ys = ys_ps.tile([128, 512], F32, tag="ys", name="ys")
                    nc.tensor.matmul(ys, xT_t, ops_slice(slot, 4),
                                     start=True, stop=True,
                                     skip_group_check=True)
                    fill_idents()
                    zv = z_pool.tile([128, 512], BF16, tag=f"zv{slot}",
                                     name=f"zv{slot}")
                    nc.vector.tensor_tensor(
                        zv[:].rearrange("t (m e) -> t m e", e=128),
                        ys[:].rearrange("t (m e) -> t m e", e=128),
                        p8n[:, pbase + slot:pbase + slot + 4]
                            .unsqueeze(2).broadcast_to([128, 4, 128]),
                        op=mybir.AluOpType.mult,
                    )
                    for j in range(4):
                        z_tiles.append(zv[:, j * 128:(j + 1) * 128])
                    slot += 4
                elif r == "N":
                    ys = ys_ps.tile([128, 512], F32, tag="ys", name="ys")
                    nc.tensor.matmul(ys, xT_t, ops_slice(slot, 4),
                                     start=True, stop=True,
                                     skip_group_check=True)
                    fill_idents()
                    for j in range(4):
                        col = p8n[:, pbase + slot + j:pbase + slot + j + 1]
                        if dve_acc is None:
                            dve_acc = acc_pool.tile([128, 128], F32,
                                                    tag="dveacc")
                            nc.vector.tensor_scalar_mul(
                                dve_acc, ys[:, j * 128:(j + 1) * 128], col)
                        else:
                            nc.vector.scalar_tensor_tensor(
                                dve_acc, ys[:, j * 128:(j + 1) * 128], col,
                                dve_acc, op0=mybir.AluOpType.mult,
                                op1=mybir.AluOpType.add)
                    slot += 4
                else:  # "S"
                    ys = ys_ps.tile([128, 512], F32, tag="ys", name="ys")
                    nc.tensor.matmul(ys, xT_t, ops_slice(slot, 4),
                                     start=True, stop=True,
                                     skip_group_check=True)
                    fill_idents()
                    for j in range(4):
                        zs = z_pool.tile([128, 128], BF16, tag=f"zs{slot + j}",
                                         name=f"zs{slot + j}")
                        nc.scalar.mul(
                            zs, ys[:, j * 128:(j + 1) * 128],
                            p8n[:, pbase + slot + j:pbase + slot + j + 1],
                        )
                        z_tiles.append(zs)
                    slot += 4
                ndrain += 2 if r in ("D", "A") else 1
                lag_n = (entry_counts[-3] if len(entry_counts) >= 3
                         else 0)
                if self_drain and lag_n > own_nid:
                    for k in range(own_nid, lag_n):
                        nc.tensor.matmul(
                            pe_acc, ident_bf, z_tiles[k],
                            start=(k == 0), stop=False,
                            skip_group_check=True,
                        )
                    own_nid = lag_n
            assert slot == M, f"plan covers {slot} slots"
            # sort by drain-engine speed: DVE z first, Pool-scaled z last
            z_dbls.sort(key=lambda kz: kz[0])
            # pair-merge the FIRST z doubles on DVE (bf16 2x adds) to cut
            # their ident count; the adds are DEFERRED into the next
            # tile's stream (emitted via closures) so they don't delay
            # this tile's ring-critical drains
            merge_ops = []
            ds = [z for kk, z in z_dbls if kk == 0]
            as_ = [z for kk, z in z_dbls if kk == 1]
            zms = []
            # pair selection: latest-ready pairs first (A-pair, then late
            # D-pairs) so early-ready z stays at the consumption front
            cand = []
            if len(ds) >= 2:
                cand.append((ds[-2], ds[-1], "D"))
            if len(as_) >= 2:
                cand.append((as_[-2], as_[-1], "A"))
            if len(ds) >= 4:
                cand.append((ds[-4], ds[-3], "D2"))
            used = set()
            for k in range(min(merges, len(cand))):
                za, zb, _ = cand[k]
                used.add(id(za)); used.add(id(zb))
                zm = z_pool.tile([128, 1024], BF16, tag=f"zm{k}",
                                 name=f"zm{k}")
                merge_ops.append((zm, za, zb))
                zms.append(zm)
            z_dbls = ([(0, z) for z in ds if id(z) not in used]
                      + [(1, z) for z in as_ if id(z) not in used]
                      + [(2, z) for z in zms])
            if not defer_merges:
                for zm, za, zb in merge_ops:
                    nc.vector.tensor_add(zm, za, zb)
                merge_ops = []
            for _, zd in z_dbls:
                for j in range(8):
                    z_tiles.append(zd[:, j * 128:(j + 1) * 128])
            if pend:
                emit_idents(pend, nid, len(pend[0]))  # flush stragglers
                emit_join(pend)
            if self_drain:
                n = len(z_tiles)
                for k in range(own_nid, n):
                    nc.tensor.matmul(
                        pe_acc, ident_bf, z_tiles[k],
                        start=(k == 0), stop=(k == n - 1),
                        skip_group_check=True,
                    )
                if own_nid >= n:
                    # all emitted with stop=False; emit a closing no-op
                    # accumulation of the last z to terminate the group
                    nc.tensor.matmul(
                        pe_acc, ident_bf, z_tiles[n - 1],
                        start=False, stop=True, skip_group_check=True,
                    )
                    raise AssertionError("unreachable: lag ensures tail")
                emit_join((None, pe_acc, dve_acc, i, []))
                return None
            return (z_tiles, pe_acc, dve_acc, i, merge_ops)

        emit_head(0)
        pend = None
        for i in range(NT):
            if i % GT == GT - 3 and i // GT + 1 < NT // GT:
                emit_head(i // GT + 1)
            pend = emit_body(i, pend, self_drain=(i == NT - 1 and last_self))
        if pend is not None:
            for zm, za, zb in pend[4]:
                nc.vector.tensor_add(zm, za, zb)
            emit_idents(pend, 0, len(pend[0]))
            emit_join(pend)

    nc.compile()
    return nc


def _get_nc(**kw):
    key = repr(sorted(kw.items()))
    if key not in _CACHE:
        _CACHE[key] = _build(**kw)
    return _CACHE[key]


def _host_prep(inputs):
    import ml_dtypes
    x = np.asarray(inputs["input_tensor"], np.float32).reshape(NTOK_TOTAL, D)
    keys = np.asarray(inputs["memory_keys"], np.float32)
    ops = np.asarray(inputs["memory_ops"], np.float32)
    wq = np.asarray(inputs["Wq"], np.float32)
    bq = np.asarray(inputs["bq"], np.float32)

    bf16 = ml_dtypes.bfloat16
    # logits = x @ K2 + bl ; K2[d, m] = sum_e Wq[e, d] keys[m, e] / sqrt(D)
    k2 = ((keys @ wq).T * SCALE).astype(bf16)                  # [D, M]
    bl = ((keys @ bq) * SCALE).astype(np.float32)              # [M]
    bl8 = np.ascontiguousarray(np.tile(bl, GT)[None, :]).astype(bf16)
    opsT = np.ascontiguousarray(
        ops.transpose(2, 0, 1).reshape(D, M * D)).astype(bf16)  # [e, (m,d)]
    xT = [
        np.ascontiguousarray(
            np.concatenate([k2, x[c * NTOK:(c + 1) * NTOK].T.astype(bf16)],
                           axis=1))
        for c in range(N_CORES)
    ]
    return xT, opsT, k2, bl8


def _run(inputs, trace=False, **build_kw):
    nc = _get_nc(**build_kw)
    xT, opsT, k2, bl8 = _host_prep(inputs)
    in_maps = [
        {"xT": xT[c], "opsT": opsT, "k2": k2, "bl8": bl8}
        for c in range(N_CORES)
    ]
    res = run_bass_kernel_spmd(
        nc, in_maps, core_ids=list(range(N_CORES)), trace=trace
    )
    out = np.concatenate(
        [res.results[c]["out"] for c in range(N_CORES)], axis=0)
    return out.reshape(B, S, D), res


def kernel(**inputs) -> np.ndarray:
    out, _ = _run(inputs, trace=False)
    return out


# revision 5
# speedup vs baseline: 1.0239x; 1.0041x over previous
"""Trainium2 Bass kernel for CoreProcessor (soft memory-slot routing).

Computation (per token t):
    q = x Wq^T + bq
    a = softmax((q keys^T) / sqrt(d))
    out = sum_m a[m] * (ops[m] @ x)

Sharding: data-parallel over the 16384 tokens across 8 cores (2048 each);
keys/ops/Wq/bq replicated.

All static layout work is done on the HOST:
  - xT bf16 [d, 64+tok] (host-transposed, with K2 = (keys Wq)^T/sqrt(d)
    packed as a 64-col header so one DMA starts the head chain),
    opsT bf16 [e, (m,d)], bl8 bias row [1, 8m] bf16. No device prologue,
    no per-tile transposes, no ACT xT copies.
  - Heads batched 8 tiles per group: one PSUM bank of logits (8 matmuls +
    1 bias matmul), one 512-wide exp, strided Z-reduce, reciprocal, and
    1/Z folded into p up front (p8n) so no output scaling remains.
  - Mains: 16 rhs chunks of 512 per tile. PSUM: two 2-bank double
    positions + one single-bank ring (shared with the logits bank via a
    common tile tag) + two rotating pe_acc banks = 8 banks.
  - Drain routes per tile (plan "AADSADSDD" = A:3 D:4 S:2 per tile,
    grid-tuned against the cost model; Pool has no PSUM port and rejects
    TensorScalarPtr, so A goes through ACT+Pool TT):
      D: DVE 1024-wide tensor_tensor (y * p bcast) -> z bf16.
      A: ACT 1024-wide copy -> bf16 SBUF; Pool TT bcast-multiply scales.
      S: ACT per-slot scaled copies (512 chunk) -> z bf16.
    One DVE pair-merge (1024-wide bf16 add, 2x mode) halves two D-chunks'
    ident count. The m-reduction is PE identity-matmuls into pe_acc,
    interleaved between the NEXT tile's mains (z is consumed
    drain-speed-sorted: singles, DVE z, Pool z, merged z) so the in-order
    PE never stalls; the per-tile join (pe_acc -> out, DVE) is emitted
    only after every ident, then DMA out.
"""

import sys

import numpy as np

sys.path.insert(0, "/opt/trn_rl_repo")

import concourse.bass as bass  # noqa: E402
import concourse.tile as tile  # noqa: E402
from concourse import bacc, mybir  # noqa: E402
from concourse.bass_utils import run_bass_kernel_spmd  # noqa: E402
from concourse.masks import make_identity  # noqa: E402

F32 = mybir.dt.float32
BF16 = mybir.dt.bfloat16

N_CORES = 8
B, S, D, M = 4, 4096, 128, 64
NTOK_TOTAL = B * S            # 16384
NTOK = NTOK_TOTAL // N_CORES  # 2048 tokens per core
NT = NTOK // 128              # 16 token tiles per core
NCHUNK = (M * D) // 512       # 16 rhs chunks of 512 (4 slots each)
GT = 8                        # tiles per head group
SCALE = 1.0 / float(np.sqrt(np.float32(D)))

_CACHE = {}


def _build(plan=None, ids_per_main=None, merges=1, zbufs=2, ybufs=2, obufs=3, last_self=False, defer_merges=True, act_join=False):
    """plan: per-parity tile route lists. Each tile's plan is a list of
    ('Dd'|'Ad', dbl_idx) doubles and ('S', sgl_idx) singles covering 16
    chunks (each Dd/Ad = 2 chunks, S = 1)."""
    from contextlib import ExitStack

    if plan is None:
        # Per-tile route tokens: D = DVE-bcast double (2 chunks),
        # A = ACT-copy + Pool-STT double, V = DVE-bcast single,
        # S = ACT slot-scaled single. Must cover 16 chunks.
        plan = {
            0: ["A", "A", "D", "S", "A", "D", "S", "D", "D"],
            1: ["A", "A", "D", "S", "A", "D", "S", "D", "D"],
        }
    # idents of the previous tile emitted after the k-th main of this tile
    if ids_per_main is None:
        ids_per_main = [5] * 6 + [2] * 6 + [1] * 4
    if isinstance(ids_per_main, int):
        ids_sched = [ids_per_main] * 24
    else:
        ids_sched = list(ids_per_main) + [0] * 24

    nc = bacc.Bacc("TRN2", target_bir_lowering=False, debug=False)

    xT_d = nc.dram_tensor("xT", [D, M + NTOK], BF16, kind="ExternalInput")
    opsT_d = nc.dram_tensor("opsT", [D, M * D], BF16, kind="ExternalInput")
    k2_d = nc.dram_tensor("k2", [D, M], BF16, kind="ExternalInput")
    bl8_d = nc.dram_tensor("bl8", [1, GT * M], BF16, kind="ExternalInput")
    out_d = nc.dram_tensor("out", [NTOK, D], F32, kind="ExternalOutput")

    with tile.TileContext(nc) as tc, ExitStack() as ctx:
        consts = ctx.enter_context(tc.tile_pool(name="consts", bufs=1))
        p_pool = ctx.enter_context(tc.tile_pool(name="p", bufs=2))
        z_pool = ctx.enter_context(tc.tile_pool(name="z", bufs=zbufs))
        y_pool = ctx.enter_context(tc.tile_pool(name="ybf", bufs=ybufs))
        acc_pool = ctx.enter_context(tc.tile_pool(name="acc", bufs=2))
        out_pool = ctx.enter_context(tc.tile_pool(name="outp", bufs=obufs))
        small = ctx.enter_context(tc.tile_pool(name="small", bufs=4))
        pa_ps = ctx.enter_context(
            tc.tile_pool(name="paps", bufs=2, space=bass.MemorySpace.PSUM)
        )
        yd_ps = ctx.enter_context(
            tc.tile_pool(name="ydps", bufs=2, space=bass.MemorySpace.PSUM)
        )
        # singles + the head's logits bank share one 2-slot ring (same tag)
        ys_ps = ctx.enter_context(
            tc.tile_pool(name="ysps", bufs=2, space=bass.MemorySpace.PSUM)
        )

        # ---- constants / inputs ----
        ident = consts.tile([128, 128], F32)
        make_identity(nc, ident)
        ident_bf = consts.tile([128, 128], BF16)
        nc.vector.tensor_copy(ident_bf, ident)
        ones_bf = consts.tile([1, 128], BF16)
        nc.vector.memset(ones_bf, 1.0)

        # k2 rides as a 64-col header on xT: one DMA starts the head chain
        kx_sb = consts.tile([D, M + NTOK], BF16)
        k2_sb = kx_sb[:, :M]
        xT_sb = kx_sb[:, M:]
        nc.sync.dma_start(kx_sb[:, :M + 256], xT_d[:, :M + 256])
        nc.sync.dma_start(kx_sb[:, M + 256:M + NTOK // 2],
                          xT_d[:, M + 256:M + NTOK // 2])
        bl8_sb = consts.tile([1, GT * M], BF16)
        nc.gpsimd.dma_start(bl8_sb, bl8_d[:])
        # opsT quarters as separate tiles for tile-granular dep tracking;
        # q2/q3 go via the (idle) gpsimd SWDGE queue in parallel with the
        # SP HWDGE queue
        MQ = M // 4
        opsT_q = [
            consts.tile([D, MQ * D], BF16, name=f"opsT_q{q}") for q in range(4)
        ]
        nc.sync.dma_start(opsT_q[0][:, :MQ * D // 2], opsT_d[:, 0:MQ * D // 2])
        nc.sync.dma_start(opsT_q[0][:, MQ * D // 2:], opsT_d[:, MQ * D // 2:MQ * D])
        nc.sync.dma_start(xT_sb[:, NTOK // 2:], xT_d[:, M + NTOK // 2:])
        nc.sync.dma_start(opsT_q[1], opsT_d[:, MQ * D:2 * MQ * D])
        nc.sync.dma_start(opsT_q[2], opsT_d[:, 2 * MQ * D:3 * MQ * D])
        nc.sync.dma_start(opsT_q[3], opsT_d[:, 3 * MQ * D:4 * MQ * D])

        def ops_slice(lo_slot, n_slots):
            """opsT columns for slots [lo_slot, lo_slot+n_slots) (must stay
            within one quarter)."""
            q = lo_slot // MQ
            base = (lo_slot - q * MQ) * D
            return opsT_q[q][:, base:base + n_slots * D]

        # ---- heads: one group of 8 tiles ----
        heads = {}

        def emit_head(g):
            lp8 = ys_ps.tile([128, GT * M], F32, tag="ys", name="lp8")
            for k in range(GT):
                t0 = (g * GT + k) * 128
                nc.tensor.matmul(
                    lp8[:, k * M:(k + 1) * M],
                    xT_sb[:, t0:t0 + 128], k2_sb,
                    start=(k == 0), stop=False, skip_group_check=True,
                )
            nc.tensor.matmul(lp8, ones_bf, bl8_sb, start=False, stop=True,
                             skip_group_check=True)
            p8 = p_pool.tile([128, GT * M], F32, tag="p8")
            nc.scalar.activation(
                p8, lp8, mybir.ActivationFunctionType.Exp, bias=0.0, scale=1.0,
            )
            zs8 = small.tile([128, GT], F32, tag="zs")
            nc.vector.tensor_reduce(
                zs8, p8[:].rearrange("t (k m) -> t k m", m=M),
                axis=mybir.AxisListType.X, op=mybir.AluOpType.add,
            )
            rz8 = small.tile([128, GT], F32, tag="rz")
            nc.vector.reciprocal(rz8, zs8)
            p8n = p_pool.tile([128, GT * M], F32, tag="p8n")
            nc.vector.tensor_tensor(
                p8n[:].rearrange("t (k m) -> t k m", m=M),
                p8[:].rearrange("t (k m) -> t k m", m=M),
                rz8[:].unsqueeze(2).broadcast_to([128, GT, M]),
                op=mybir.AluOpType.mult,
            )
            for k in range(GT):
                heads[g * GT + k] = (p8n, k)

        def emit_idents(pend, lo, hi):
            z_tiles, pe_acc = pend[0], pend[1]
            n = len(z_tiles)
            for k in range(min(lo, n), min(hi, n)):
                nc.tensor.matmul(
                    pe_acc, ident_bf, z_tiles[k],
                    start=(k == 0), stop=(k == n - 1),
                    skip_group_check=True,
                )

        def emit_join(pend):
            """out_tile = pe_acc (+ dve_acc), then DMA."""
            _, pe_acc, dve_acc, ti = pend[:4]
            out_t = out_pool.tile([128, 128], F32)
            if dve_acc is not None:
                nc.vector.scalar_tensor_tensor(
                    out_t, pe_acc, 1.0, dve_acc,
                    op0=mybir.AluOpType.mult, op1=mybir.AluOpType.add,
                )
            elif act_join:
                nc.scalar.copy(out_t, pe_acc)
            else:
                nc.vector.tensor_copy(out_t, pe_acc)
            nc.sync.dma_start(out_d[ti * 128:(ti + 1) * 128, :], out_t)

        def emit_body(i, pend, self_drain=False):
            p8n, kk = heads.pop(i)
            pbase = kk * M
            xT_t = xT_sb[:, i * 128:(i + 1) * 128]
            pe_acc = pa_ps.tile([128, 128], F32, tag="pacc")
            pool_acc = None
            dve_acc = None
            z_tiles = []
            z_dbls = []
            routes = plan.get(i, plan[i % 2]) if isinstance(plan, dict) \
                else plan[i % 2]
            slot = 0
            nid = 0  # idents of prev emitted so far
            own_nid = 0
            ndrain = 0

            nmain = 0

            def fill_idents():
                nonlocal nid, nmain
                k = ids_sched[nmain]
                nmain += 1
                if pend:
                    emit_idents(pend, nid, nid + k)
                    nid += k

            entry_counts = []
            n_entry = 0
            for r in routes:
                n_entry += 1
                if pend and n_entry == 2:
                    for zm, za, zb in pend[4]:
                        nc.vector.tensor_add(zm, za, zb)
                entry_counts.append(len(z_tiles))
                if r == "D":
                    yd = yd_ps.tile([128, 1024], F32, tag="ydd")
                    nc.tensor.matmul(yd[:, :512], xT_t, ops_slice(slot, 4),
                                     start=True, stop=True,
                                     skip_group_check=True)
                    fill_idents()
                    nc.tensor.matmul(yd[:, 512:], xT_t, ops_slice(slot + 4, 4),
                                     start=True, stop=True,
                                     skip_group_check=True)
                    fill_idents()
                    zd = z_pool.tile([128, 1024], BF16, tag=f"zd{slot}",
                                     name=f"zd{slot}")
                    nc.vector.tensor_tensor(
                        zd[:].rearrange("t (m e) -> t m e", e=128),
                        yd[:].rearrange("t (m e) -> t m e", e=128),
                        p8n[:, pbase + slot:pbase + slot + 8]
                            .unsqueeze(2).broadcast_to([128, 8, 128]),
                        op=mybir.AluOpType.mult,
                    )
                    if self_drain:
                        for j in range(8):
                            z_tiles.append(zd[:, j * 128:(j + 1) * 128])
                    else:
                        z_dbls.append((0, zd))
                    slot += 8
                elif r == "P":
                    # A-pair: two doubles copied into one 2048-wide bf16
                    # buffer, ONE Pool TT scales all 16 slots
                    yb2 = y_pool.tile([128, 2048], BF16, tag=f"yb2{slot}",
                                      name=f"yb2{slot}")
                    for h in range(2):
                        yd = yd_ps.tile([128, 1024], F32, tag="ydd")
                        nc.tensor.matmul(yd[:, :512], xT_t,
                                         ops_slice(slot + 8 * h, 4),
                                         start=True, stop=True,
                                         skip_group_check=True)
                        fill_idents()
                        nc.tensor.matmul(yd[:, 512:], xT_t,
                                         ops_slice(slot + 8 * h + 4, 4),
                                         start=True, stop=True,
                                         skip_group_check=True)
                        fill_idents()
                        nc.scalar.copy(yb2[:, h * 1024:(h + 1) * 1024], yd)
                    za2 = z_pool.tile([128, 2048], BF16, tag=f"za2{slot}",
                                      name=f"za2{slot}")
                    nc.gpsimd.tensor_tensor(
                        za2[:].rearrange("t (m e) -> t m e", e=128),
                        yb2[:].rearrange("t (m e) -> t m e", e=128),
                        p8n[:, pbase + slot:pbase + slot + 16]
                            .unsqueeze(2).broadcast_to([128, 16, 128]),
                        op=mybir.AluOpType.mult,
                    )
                    z_dbls.append((1, za2[:, :1024]))
                    z_dbls.append((1, za2[:, 1024:]))
                    slot += 16
                elif r == "A":
                    yd = yd_ps.tile([128, 1024], F32, tag="ydd")
                    nc.tensor.matmul(yd[:, :512], xT_t, ops_slice(slot, 4),
                                     start=True, stop=True,
                                     skip_group_check=True)
                    fill_idents()
                    nc.tensor.matmul(yd[:, 512:], xT_t, ops_slice(slot + 4, 4),
                                     start=True, stop=True,
                                     skip_group_check=True)
                    fill_idents()
                    yb = y_pool.tile([128, 1024], BF16, tag=f"yb{slot}",
                                     name=f"yb{slot}")
                    nc.scalar.copy(yb, yd)
                    za = z_pool.tile([128, 1024], BF16, tag=f"za{slot}",
                                     name=f"za{slot}")
                    nc.gpsimd.tensor_tensor(
                        za[:].rearrange("t (m e) -> t m e", e=128),
                        yb[:].rearrange("t (m e) -> t m e", e=128),
                        p8n[:, pbase + slot:pbase + slot + 8]
                            .unsqueeze(2).broadcast_to([128, 8, 128]),
                        op=mybir.AluOpType.mult,
                    )
                    if self_drain:
                        for j in range(8):
                            z_tiles.append(za[:, j * 128:(j + 1) * 128])
                    else:
                        z_dbls.append((1, za))
                    slot += 8
                elif r == "V":
                    ys = ys_ps.tile([128, 512], F32, tag="ys", name="ys")
                    nc.tensor.matmul(ys, xT_t, ops_slice(slot, 4),
                                     start=True, stop=True,
                                     skip_group_check=True)
                    fill_idents()
                    zv = z_pool.tile([128, 512], BF16, tag=f"zv{slot}",
                                     name=f"zv{slot}")
                    nc.vector.tensor_tensor(
                        zv[:].rearrange("t (m e) -> t m e", e=128),
                        ys[:].rearrange("t (m e) -> t m e", e=128),
                        p8n[:, pbase + slot:pbase + slot + 4]
                            .unsqueeze(2).broadcast_to([128, 4, 128]),
                        op=mybir.AluOpType.mult,
                    )
                    for j in range(4):
                        z_tiles.append(zv[:, j * 128:(j + 1) * 128])
                    slot += 4
                elif r == "N":
                    ys = ys_ps.tile([128, 512], F32, tag="ys", name="ys")
                    nc.tensor.matmul(ys, xT_t, ops_slice(slot, 4),
                                     start=True, stop=True,
                                     skip_group_check=True)
                    fill_idents()
                    for j in range(4):
                        col = p8n[:, pbase + slot + j:pbase + slot + j + 1]
                        if dve_acc is None:
                            dve_acc = acc_pool.tile([128, 128], F32,
                                                    tag="dveacc")
                            nc.vector.tensor_scalar_mul(
                                dve_acc, ys[:, j * 128:(j + 1) * 128], col)
                        else:
                            nc.vector.scalar_tensor_tensor(
                                dve_acc, ys[:, j * 128:(j + 1) * 128], col,
                                dve_acc, op0=mybir.AluOpType.mult,
                                op1=mybir.AluOpType.add)
                    slot += 4
                else:  # "S"
                    ys = ys_ps.tile([128, 512], F32, tag="ys", name="ys")
                    nc.tensor.matmul(ys, xT_t, ops_slice(slot, 4),
                                     start=True, stop=True,
                                     skip_group_check=True)
                    fill_idents()
                    for j in range(4):
                        zs = z_pool.tile([128, 128], BF16, tag=f"zs{slot + j}",
                                         name=f"zs{slot + j}")
                        nc.scalar.mul(
                            zs, ys[:, j * 128:(j + 1) * 128],
                            p8n[:, pbase + slot + j:pbase + slot + j + 1],
                        )
                        z_tiles.append(zs)
                    slot += 4
                ndrain += 2 if r in ("D", "A") else 1
                lag_n = (entry_counts[-3] if len(entry_counts) >= 3
                         else 0)
                if self_drain and lag_n > own_nid:
                    for k in range(own_nid, lag_n):
                        nc.tensor.matmul(
                            pe_acc, ident_bf, z_tiles[k],
                            start=(k == 0), stop=False,
                            skip_group_check=True,
                        )
                    own_nid = lag_n
            assert slot == M, f"plan covers {slot} slots"
            # sort by drain-engine speed: DVE z first, Pool-scaled z last
            z_dbls.sort(key=lambda kz: kz[0])
            # pair-merge the FIRST z doubles on DVE (bf16 2x adds) to cut
            # their ident count; the adds are DEFERRED into the next
            # tile's stream (emitted via closures) so they don't delay
            # this tile's ring-critical drains
            merge_ops = []
            ds = [z for kk, z in z_dbls if kk == 0]
            as_ = [z for kk, z in z_dbls if kk == 1]
            zms = []
            # pair selection: latest-ready pairs first (A-pair, then late
            # D-pairs) so early-ready z stays at the consumption front
            cand = []
            if len(ds) >= 2:
                cand.append((ds[-2], ds[-1], "D"))
            if len(as_) >= 2:
                cand.append((as_[-2], as_[-1], "A"))
            if len(ds) >= 4:
                cand.append((ds[-4], ds[-3], "D2"))
            used = set()
            for k in range(min(merges, len(cand))):
                za, zb, _ = cand[k]
                used.add(id(za)); used.add(id(zb))
                zm = z_pool.tile([128, 1024], BF16, tag=f"zm{k}",
                                 name=f"zm{k}")
                merge_ops.append((zm, za, zb))
                zms.append(zm)
            z_dbls = ([(0, z) for z in ds if id(z) not in used]
                      + [(1, z) for z in as_ if id(z) not in used]
                      + [(2, z) for z in zms])
            if not defer_merges:
                for zm, za, zb in merge_ops:
                    nc.vector.tensor_add(zm, za, zb)
                merge_ops = []
            for _, zd in z_dbls:
                for j in range(8):
                    z_tiles.append(zd[:, j * 128:(j + 1) * 128])
            if pend:
                emit_idents(pend, nid, len(pend[0]))  # flush stragglers
                emit_join(pend)
            if self_drain:
                n = len(z_tiles)
                for k in range(own_nid, n):
                    nc.tensor.matmul(
                        pe_acc, ident_bf, z_tiles[k],
                        start=(k == 0), stop=(k == n - 1),
                        skip_group_check=True,
                    )
                if own_nid >= n:
                    # all emitted with stop=False; emit a closing no-op
                    # accumulation of the last z to terminate the group
                    nc.tensor.matmul(
                        pe_acc, ident_bf, z_tiles[n - 1],
                        start=False, stop=True, skip_group_check=True,
                    )
                    raise AssertionError("unreachable: lag ensures tail")
                emit_join((None, pe_acc, dve_acc, i, []))
                return None
            return (z_tiles, pe_acc, dve_acc, i, merge_ops)

        emit_head(0)
        pend = None
        for i in range(NT):
            if i % GT == GT - 3 and i // GT + 1 < NT // GT:
                emit_head(i // GT + 1)
            pend = emit_body(i, pend, self_drain=(i == NT - 1 and last_self))
        if pend is not None:
            for zm, za, zb in pend[4]:
                nc.vector.tensor_add(zm, za, zb)
            emit_idents(pend, 0, len(pend[0]))
            emit_join(pend)

    nc.compile()
    return nc


def _get_nc(**kw):
    key = repr(sorted(kw.items()))
    if key not in _CACHE:
        _CACHE[key] = _build(**kw)
    return _CACHE[key]


def _host_prep(inputs):
    import ml_dtypes
    x = np.asarray(inputs["input_tensor"], np.float32).reshape(NTOK_TOTAL, D)
    keys = np.asarray(inputs["memory_keys"], np.float32)
    ops = np.asarray(inputs["memory_ops"], np.float32)
    wq = np.asarray(inputs["Wq"], np.float32)
    bq = np.asarray(inputs["bq"], np.float32)

    bf16 = ml_dtypes.bfloat16
    # logits = x @ K2 + bl ; K2[d, m] = sum_e Wq[e, d] keys[m, e] / sqrt(D)
    k2 = ((keys @ wq).T * SCALE).astype(bf16)                  # [D, M]
    bl = ((keys @ bq) * SCALE).astype(np.float32)              # [M]
    bl8 = np.ascontiguousarray(np.tile(bl, GT)[None, :]).astype(bf16)
    opsT = np.ascontiguousarray(
        ops.transpose(2, 0, 1).reshape(D, M * D)).astype(bf16)  # [e, (m,d)]
    xT = [
        np.ascontiguousarray(
            np.concatenate([k2, x[c * NTOK:(c + 1) * NTOK].T.astype(bf16)],
                           axis=1))
        for c in range(N_CORES)
    ]
    return xT, opsT, k2, bl8


def _run(inputs, trace=False, **build_kw):
    nc = _get_nc(**build_kw)
    xT, opsT, k2, bl8 = _host_prep(inputs)
    in_maps = [
        {"xT": xT[c], "opsT": opsT, "k2": k2, "bl8": bl8}
        for c in range(N_CORES)
    ]
    res = run_bass_kernel_spmd(
        nc, in_maps, core_ids=list(range(N_CORES)), trace=trace
    )
    out = np.concatenate(
        [res.results[c]["out"] for c in range(N_CORES)], axis=0)
    return out.reshape(B, S, D), res


def kernel(**inputs) -> np.ndarray:
    out, _ = _run(inputs, trace=False)
    return out


# revision 6
# speedup vs baseline: 1.0245x; 1.0007x over previous
"""Trainium2 Bass kernel for CoreProcessor (soft memory-slot routing).

Computation (per token t):
    q = x Wq^T + bq
    a = softmax((q keys^T) / sqrt(d))
    out = sum_m a[m] * (ops[m] @ x)

Sharding: data-parallel over the 16384 tokens across 8 cores (2048 each);
keys/ops/Wq/bq replicated.

All static layout work is done on the HOST:
  - xT bf16 [d, 64+tok] (host-transposed, with K2 = (keys Wq)^T/sqrt(d)
    packed as a 64-col header so one DMA starts the head chain),
    opsT bf16 [e, (m,d)], bl8 bias row [1, 8m] bf16. No device prologue,
    no per-tile transposes, no ACT xT copies.
  - Heads batched 8 tiles per group: one PSUM bank of logits (8 matmuls +
    1 bias matmul), one 512-wide exp, strided Z-reduce, reciprocal, and
    1/Z folded into p up front (p8n) so no output scaling remains.
  - Mains: 16 rhs chunks of 512 per tile. PSUM: two 2-bank double
    positions + one single-bank ring (shared with the logits bank via a
    common tile tag) + two rotating pe_acc banks = 8 banks.
  - Drain routes per tile (plan "AADSADSDD" = A:3 D:4 S:2 per tile,
    grid-tuned against the cost model; Pool has no PSUM port and rejects
    TensorScalarPtr, so A goes through ACT+Pool TT):
      D: DVE 1024-wide tensor_tensor (y * p bcast) -> z bf16.
      A: ACT 1024-wide copy -> bf16 SBUF; Pool TT bcast-multiply scales.
      S: ACT per-slot scaled copies (512 chunk) -> z bf16.
    One DVE pair-merge (1024-wide bf16 add, 2x mode) halves two D-chunks'
    ident count. The m-reduction is PE identity-matmuls into pe_acc,
    interleaved between the NEXT tile's mains (z is consumed
    drain-speed-sorted: singles, DVE z, Pool z, merged z) so the in-order
    PE never stalls; the per-tile join (pe_acc -> out, DVE) is emitted
    only after every ident, then DMA out.
"""

import sys

import numpy as np

sys.path.insert(0, "/opt/trn_rl_repo")

import concourse.bass as bass  # noqa: E402
import concourse.tile as tile  # noqa: E402
from concourse import bacc, mybir  # noqa: E402
from concourse.bass_utils import run_bass_kernel_spmd  # noqa: E402
from concourse.masks import make_identity  # noqa: E402

F32 = mybir.dt.float32
BF16 = mybir.dt.bfloat16

N_CORES = 8
B, S, D, M = 4, 4096, 128, 64
NTOK_TOTAL = B * S            # 16384
NTOK = NTOK_TOTAL // N_CORES  # 2048 tokens per core
NT = NTOK // 128              # 16 token tiles per core
NCHUNK = (M * D) // 512       # 16 rhs chunks of 512 (4 slots each)
GT = 8                        # tiles per head group
SCALE = 1.0 / float(np.sqrt(np.float32(D)))

_CACHE = {}


def _build(plan=None, ids_per_main=None, merges=1, zbufs=2, ybufs=2, obufs=3, last_self=False, defer_merges=True, act_join=False, n_warm=28, merge_entry=2):
    """plan: per-parity tile route lists. Each tile's plan is a list of
    ('Dd'|'Ad', dbl_idx) doubles and ('S', sgl_idx) singles covering 16
    chunks (each Dd/Ad = 2 chunks, S = 1)."""
    from contextlib import ExitStack

    if plan is None:
        # Per-tile route tokens: D = DVE-bcast double (2 chunks),
        # A = ACT-copy + Pool-STT double, V = DVE-bcast single,
        # S = ACT slot-scaled single. Must cover 16 chunks.
        plan = {
            0: ["A", "A", "D", "S", "A", "D", "S", "D", "D"],
            1: ["A", "A", "D", "S", "A", "D", "S", "D", "D"],
        }
    # idents of the previous tile emitted after the k-th main of this tile
    if ids_per_main is None:
        ids_per_main = [5] * 6 + [2] * 6 + [1] * 4
    if isinstance(ids_per_main, int):
        ids_sched = [ids_per_main] * 24
    else:
        ids_sched = list(ids_per_main) + [0] * 24

    nc = bacc.Bacc("TRN2", target_bir_lowering=False, debug=False)

    xT_d = nc.dram_tensor("xT", [D, M + NTOK], BF16, kind="ExternalInput")
    opsT_d = nc.dram_tensor("opsT", [D, M * D], BF16, kind="ExternalInput")
    k2_d = nc.dram_tensor("k2", [D, M], BF16, kind="ExternalInput")
    bl8_d = nc.dram_tensor("bl8", [1, GT * M], BF16, kind="ExternalInput")
    out_d = nc.dram_tensor("out", [NTOK, D], F32, kind="ExternalOutput")

    with tile.TileContext(nc) as tc, ExitStack() as ctx:
        consts = ctx.enter_context(tc.tile_pool(name="consts", bufs=1))
        p_pool = ctx.enter_context(tc.tile_pool(name="p", bufs=2))
        z_pool = ctx.enter_context(tc.tile_pool(name="z", bufs=zbufs))
        y_pool = ctx.enter_context(tc.tile_pool(name="ybf", bufs=ybufs))
        acc_pool = ctx.enter_context(tc.tile_pool(name="acc", bufs=2))
        out_pool = ctx.enter_context(tc.tile_pool(name="outp", bufs=obufs))
        small = ctx.enter_context(tc.tile_pool(name="small", bufs=4))
        pa_ps = ctx.enter_context(
            tc.tile_pool(name="paps", bufs=2, space=bass.MemorySpace.PSUM)
        )
        yd_ps = ctx.enter_context(
            tc.tile_pool(name="ydps", bufs=2, space=bass.MemorySpace.PSUM)
        )
        # singles + the head's logits bank share one 2-slot ring (same tag)
        ys_ps = ctx.enter_context(
            tc.tile_pool(name="ysps", bufs=2, space=bass.MemorySpace.PSUM)
        )

        # ---- constants / inputs ----
        ident = consts.tile([128, 128], F32)
        make_identity(nc, ident)
        ident_bf = consts.tile([128, 128], BF16)
        nc.vector.tensor_copy(ident_bf, ident)
        ones_bf = consts.tile([1, 128], BF16)
        nc.vector.memset(ones_bf, 1.0)

        # PE warm-up: dummy matmuls from t~0 so the p-state ramp (2x slow
        # until 3us of continuous busy) completes before real work arrives.
        # They rotate through the pe_acc ring, which real work first needs
        # only ~1.5 tiles in.
        for w in range(n_warm):
            wrm = pa_ps.tile([128, 128], F32, tag="pacc", name="warm")
            nc.tensor.matmul(wrm, ident_bf, ident_bf, start=True, stop=True,
                             skip_group_check=True)

        # k2 rides as a 64-col header on xT: one DMA starts the head chain
        kx_sb = consts.tile([D, M + NTOK], BF16)
        k2_sb = kx_sb[:, :M]
        xT_sb = kx_sb[:, M:]
        nc.sync.dma_start(kx_sb[:, :M + 256], xT_d[:, :M + 256])
        nc.sync.dma_start(kx_sb[:, M + 256:M + NTOK // 2],
                          xT_d[:, M + 256:M + NTOK // 2])
        bl8_sb = consts.tile([1, GT * M], BF16)
        nc.gpsimd.dma_start(bl8_sb, bl8_d[:])
        # opsT quarters as separate tiles for tile-granular dep tracking;
        # q2/q3 go via the (idle) gpsimd SWDGE queue in parallel with the
        # SP HWDGE queue
        MQ = M // 4
        opsT_q = [
            consts.tile([D, MQ * D], BF16, name=f"opsT_q{q}") for q in range(4)
        ]
        nc.sync.dma_start(opsT_q[0][:, :MQ * D // 2], opsT_d[:, 0:MQ * D // 2])
        nc.sync.dma_start(opsT_q[0][:, MQ * D // 2:], opsT_d[:, MQ * D // 2:MQ * D])
        nc.sync.dma_start(xT_sb[:, NTOK // 2:], xT_d[:, M + NTOK // 2:])
        nc.sync.dma_start(opsT_q[1], opsT_d[:, MQ * D:2 * MQ * D])
        nc.sync.dma_start(opsT_q[2], opsT_d[:, 2 * MQ * D:3 * MQ * D])
        nc.sync.dma_start(opsT_q[3], opsT_d[:, 3 * MQ * D:4 * MQ * D])

        def ops_slice(lo_slot, n_slots):
            """opsT columns for slots [lo_slot, lo_slot+n_slots) (must stay
            within one quarter)."""
            q = lo_slot // MQ
            base = (lo_slot - q * MQ) * D
            return opsT_q[q][:, base:base + n_slots * D]

        # ---- heads: one group of 8 tiles ----
        heads = {}

        def emit_head(g):
            lp8 = ys_ps.tile([128, GT * M], F32, tag="ys", name="lp8")
            for k in range(GT):
                t0 = (g * GT + k) * 128
                nc.tensor.matmul(
                    lp8[:, k * M:(k + 1) * M],
                    xT_sb[:, t0:t0 + 128], k2_sb,
                    start=(k == 0), stop=False, skip_group_check=True,
                )
            nc.tensor.matmul(lp8, ones_bf, bl8_sb, start=False, stop=True,
                             skip_group_check=True)
            p8 = p_pool.tile([128, GT * M], F32, tag="p8")
            nc.scalar.activation(
                p8, lp8, mybir.ActivationFunctionType.Exp, bias=0.0, scale=1.0,
            )
            zs8 = small.tile([128, GT], F32, tag="zs")
            nc.vector.tensor_reduce(
                zs8, p8[:].rearrange("t (k m) -> t k m", m=M),
                axis=mybir.AxisListType.X, op=mybir.AluOpType.add,
            )
            rz8 = small.tile([128, GT], F32, tag="rz")
            nc.vector.reciprocal(rz8, zs8)
            p8n = p_pool.tile([128, GT * M], F32, tag="p8n")
            nc.vector.tensor_tensor(
                p8n[:].rearrange("t (k m) -> t k m", m=M),
                p8[:].rearrange("t (k m) -> t k m", m=M),
                rz8[:].unsqueeze(2).broadcast_to([128, GT, M]),
                op=mybir.AluOpType.mult,
            )
            for k in range(GT):
                heads[g * GT + k] = (p8n, k)

        def emit_idents(pend, lo, hi):
            z_tiles, pe_acc = pend[0], pend[1]
            n = len(z_tiles)
            for k in range(min(lo, n), min(hi, n)):
                nc.tensor.matmul(
                    pe_acc, ident_bf, z_tiles[k],
                    start=(k == 0), stop=(k == n - 1),
                    skip_group_check=True,
                )

        def emit_join(pend):
            """out_tile = pe_acc (+ dve_acc), then DMA."""
            _, pe_acc, dve_acc, ti = pend[:4]
            out_t = out_pool.tile([128, 128], F32)
            if dve_acc is not None:
                nc.vector.scalar_tensor_tensor(
                    out_t, pe_acc, 1.0, dve_acc,
                    op0=mybir.AluOpType.mult, op1=mybir.AluOpType.add,
                )
            elif act_join:
                nc.scalar.copy(out_t, pe_acc)
            else:
                nc.vector.tensor_copy(out_t, pe_acc)
            nc.sync.dma_start(out_d[ti * 128:(ti + 1) * 128, :], out_t)

        def emit_body(i, pend, self_drain=False):
            p8n, kk = heads.pop(i)
            pbase = kk * M
            xT_t = xT_sb[:, i * 128:(i + 1) * 128]
            pe_acc = pa_ps.tile([128, 128], F32, tag="pacc")
            pool_acc = None
            dve_acc = None
            z_tiles = []
            z_dbls = []
            routes = plan.get(i, plan[i % 2]) if isinstance(plan, dict) \
                else plan[i % 2]
            slot = 0
            nid = 0  # idents of prev emitted so far
            own_nid = 0
            ndrain = 0

            nmain = 0

            def fill_idents():
                nonlocal nid, nmain
                k = ids_sched[nmain]
                nmain += 1
                if pend:
                    emit_idents(pend, nid, nid + k)
                    nid += k

            entry_counts = []
            n_entry = 0
            for r in routes:
                n_entry += 1
                if pend and n_entry == merge_entry:
                    for zm, za, zb in pend[4]:
                        nc.vector.tensor_add(zm, za, zb)
                entry_counts.append(len(z_tiles))
                if r == "D":
                    yd = yd_ps.tile([128, 1024], F32, tag="ydd")
                    nc.tensor.matmul(yd[:, :512], xT_t, ops_slice(slot, 4),
                                     start=True, stop=True,
                                     skip_group_check=True)
                    fill_idents()
                    nc.tensor.matmul(yd[:, 512:], xT_t, ops_slice(slot + 4, 4),
                                     start=True, stop=True,
                                     skip_group_check=True)
                    fill_idents()
                    zd = z_pool.tile([128, 1024], BF16, tag=f"zd{slot}",
                                     name=f"zd{slot}")
                    nc.vector.tensor_tensor(
                        zd[:].rearrange("t (m e) -> t m e", e=128),
                        yd[:].rearrange("t (m e) -> t m e", e=128),
                        p8n[:, pbase + slot:pbase + slot + 8]
                            .unsqueeze(2).broadcast_to([128, 8, 128]),
                        op=mybir.AluOpType.mult,
                    )
                    if self_drain:
                        for j in range(8):
                            z_tiles.append(zd[:, j * 128:(j + 1) * 128])
                    else:
                        z_dbls.append((0, zd))
                    slot += 8
                elif r == "P":
                    # A-pair: two doubles copied into one 2048-wide bf16
                    # buffer, ONE Pool TT scales all 16 slots
                    yb2 = y_pool.tile([128, 2048], BF16, tag=f"yb2{slot}",
                                      name=f"yb2{slot}")
                    for h in range(2):
                        yd = yd_ps.tile([128, 1024], F32, tag="ydd")
                        nc.tensor.matmul(yd[:, :512], xT_t,
                                         ops_slice(slot + 8 * h, 4),
                                         start=True, stop=True,
                                         skip_group_check=True)
                        fill_idents()
                        nc.tensor.matmul(yd[:, 512:], xT_t,
                                         ops_slice(slot + 8 * h + 4, 4),
                                         start=True, stop=True,
                                         skip_group_check=True)
                        fill_idents()
                        nc.scalar.copy(yb2[:, h * 1024:(h + 1) * 1024], yd)
                    za2 = z_pool.tile([128, 2048], BF16, tag=f"za2{slot}",
                                      name=f"za2{slot}")
                    nc.gpsimd.tensor_tensor(
                        za2[:].rearrange("t (m e) -> t m e", e=128),
                        yb2[:].rearrange("t (m e) -> t m e", e=128),
                        p8n[:, pbase + slot:pbase + slot + 16]
                            .unsqueeze(2).broadcast_to([128, 16, 128]),
                        op=mybir.AluOpType.mult,
                    )
                    z_dbls.append((1, za2[:, :1024]))
                    z_dbls.append((1, za2[:, 1024:]))
                    slot += 16
                elif r == "B":
                    # like A but the bcast scale runs on DVE (SBUF, 1x)
                    yd = yd_ps.tile([128, 1024], F32, tag="ydd")
                    nc.tensor.matmul(yd[:, :512], xT_t, ops_slice(slot, 4),
                                     start=True, stop=True,
                                     skip_group_check=True)
                    fill_idents()
                    nc.tensor.matmul(yd[:, 512:], xT_t, ops_slice(slot + 4, 4),
                                     start=True, stop=True,
                                     skip_group_check=True)
                    fill_idents()
                    yb = y_pool.tile([128, 1024], BF16, tag=f"yb{slot}",
                                     name=f"yb{slot}")
                    nc.scalar.copy(yb, yd)
                    za = z_pool.tile([128, 1024], BF16, tag=f"za{slot}",
                                     name=f"za{slot}")
                    nc.vector.tensor_tensor(
                        za[:].rearrange("t (m e) -> t m e", e=128),
                        yb[:].rearrange("t (m e) -> t m e", e=128),
                        p8n[:, pbase + slot:pbase + slot + 8]
                            .unsqueeze(2).broadcast_to([128, 8, 128]),
                        op=mybir.AluOpType.mult,
                    )
                    z_dbls.append((1, za))
                    slot += 8
                elif r == "A":
                    yd = yd_ps.tile([128, 1024], F32, tag="ydd")
                    nc.tensor.matmul(yd[:, :512], xT_t, ops_slice(slot, 4),
                                     start=True, stop=True,
                                     skip_group_check=True)
                    fill_idents()
                    nc.tensor.matmul(yd[:, 512:], xT_t, ops_slice(slot + 4, 4),
                                     start=True, stop=True,
                                     skip_group_check=True)
                    fill_idents()
                    yb = y_pool.tile([128, 1024], BF16, tag=f"yb{slot}",
                                     name=f"yb{slot}")
                    nc.scalar.copy(yb, yd)
                    za = z_pool.tile([128, 1024], BF16, tag=f"za{slot}",
                                     name=f"za{slot}")
                    nc.gpsimd.tensor_tensor(
                        za[:].rearrange("t (m e) -> t m e", e=128),
                        yb[:].rearrange("t (m e) -> t m e", e=128),
                        p8n[:, pbase + slot:pbase + slot + 8]
                            .unsqueeze(2).broadcast_to([128, 8, 128]),
                        op=mybir.AluOpType.mult,
                    )
                    if self_drain:
                        for j in range(8):
                            z_tiles.append(za[:, j * 128:(j + 1) * 128])
                    else:
                        z_dbls.append((1, za))
                    slot += 8
                elif r == "V":
                    ys = ys_ps.tile([128, 512], F32, tag="ys", name="ys")
                    nc.tensor.matmul(ys, xT_t, ops_slice(slot, 4),
                                     start=True, stop=True,
                                     skip_group_check=True)
                    fill_idents()
                    zv = z_pool.tile([128, 512], BF16, tag=f"zv{slot}",
                                     name=f"zv{slot}")
                    nc.vector.tensor_tensor(
                        zv[:].rearrange("t (m e) -> t m e", e=128),
                        ys[:].rearrange("t (m e) -> t m e", e=128),
                        p8n[:, pbase + slot:pbase + slot + 4]
                            .unsqueeze(2).broadcast_to([128, 4, 128]),
                        op=mybir.AluOpType.mult,
                    )
                    for j in range(4):
                        z_tiles.append(zv[:, j * 128:(j + 1) * 128])
                    slot += 4
                elif r == "N":
                    ys = ys_ps.tile([128, 512], F32, tag="ys", name="ys")
                    nc.tensor.matmul(ys, xT_t, ops_slice(slot, 4),
                                     start=True, stop=True,
                                     skip_group_check=True)
                    fill_idents()
                    for j in range(4):
                        col = p8n[:, pbase + slot + j:pbase + slot + j + 1]
                        if dve_acc is None:
                            dve_acc = acc_pool.tile([128, 128], F32,
                                                    tag="dveacc")
                            nc.vector.tensor_scalar_mul(
                                dve_acc, ys[:, j * 128:(j + 1) * 128], col)
                        else:
                            nc.vector.scalar_tensor_tensor(
                                dve_acc, ys[:, j * 128:(j + 1) * 128], col,
                                dve_acc, op0=mybir.AluOpType.mult,
                                op1=mybir.AluOpType.add)
                    slot += 4
                else:  # "S"
                    ys = ys_ps.tile([128, 512], F32, tag="ys", name="ys")
                    nc.tensor.matmul(ys, xT_t, ops_slice(slot, 4),
                                     start=True, stop=True,
                                     skip_group_check=True)
                    fill_idents()
                    for j in range(4):
                        zs = z_pool.tile([128, 128], BF16, tag=f"zs{slot + j}",
                                         name=f"zs{slot + j}")
                        nc.scalar.mul(
                            zs, ys[:, j * 128:(j + 1) * 128],
                            p8n[:, pbase + slot + j:pbase + slot + j + 1],
                        )
                        z_tiles.append(zs)
                    slot += 4
                ndrain += 2 if r in ("D", "A") else 1
                lag_n = (entry_counts[-3] if len(entry_counts) >= 3
                         else 0)
                if self_drain and lag_n > own_nid:
                    for k in range(own_nid, lag_n):
                        nc.tensor.matmul(
                            pe_acc, ident_bf, z_tiles[k],
                            start=(k == 0), stop=False,
                            skip_group_check=True,
                        )
                    own_nid = lag_n
            assert slot == M, f"plan covers {slot} slots"
            # sort by drain-engine speed: DVE z first, Pool-scaled z last
            z_dbls.sort(key=lambda kz: kz[0])
            # pair-merge the FIRST z doubles on DVE (bf16 2x adds) to cut
            # their ident count; the adds are DEFERRED into the next
            # tile's stream (emitted via closures) so they don't delay
            # this tile's ring-critical drains
            merge_ops = []
            ds = [z for kk, z in z_dbls if kk == 0]
            as_ = [z for kk, z in z_dbls if kk == 1]
            zms = []
            # pair selection: latest-ready pairs first (A-pair, then late
            # D-pairs) so early-ready z stays at the consumption front
            cand = []
            if len(ds) >= 2:
                cand.append((ds[-2], ds[-1], "D"))
            if len(as_) >= 2:
                cand.append((as_[-2], as_[-1], "A"))
            if len(ds) >= 4:
                cand.append((ds[-4], ds[-3], "D2"))
            used = set()
            for k in range(min(merges, len(cand))):
                za, zb, _ = cand[k]
                used.add(id(za)); used.add(id(zb))
                zm = z_pool.tile([128, 1024], BF16, tag=f"zm{k}",
                                 name=f"zm{k}")
                merge_ops.append((zm, za, zb))
                zms.append(zm)
            z_dbls = ([(0, z) for z in ds if id(z) not in used]
                      + [(1, z) for z in as_ if id(z) not in used]
                      + [(2, z) for z in zms])
            if not defer_merges:
                for zm, za, zb in merge_ops:
                    nc.vector.tensor_add(zm, za, zb)
                merge_ops = []
            for _, zd in z_dbls:
                for j in range(8):
                    z_tiles.append(zd[:, j * 128:(j + 1) * 128])
            if pend:
                emit_idents(pend, nid, len(pend[0]))  # flush stragglers
                emit_join(pend)
            if self_drain:
                n = len(z_tiles)
                for k in range(own_nid, n):
                    nc.tensor.matmul(
                        pe_acc, ident_bf, z_tiles[k],
                        start=(k == 0), stop=(k == n - 1),
                        skip_group_check=True,
                    )
                if own_nid >= n:
                    # all emitted with stop=False; emit a closing no-op
                    # accumulation of the last z to terminate the group
                    nc.tensor.matmul(
                        pe_acc, ident_bf, z_tiles[n - 1],
                        start=False, stop=True, skip_group_check=True,
                    )
                    raise AssertionError("unreachable: lag ensures tail")
                emit_join((None, pe_acc, dve_acc, i, []))
                return None
            return (z_tiles, pe_acc, dve_acc, i, merge_ops)

        emit_head(0)
        pend = None
        for i in range(NT):
            if i % GT == GT - 3 and i // GT + 1 < NT // GT:
                emit_head(i // GT + 1)
            pend = emit_body(i, pend, self_drain=(i == NT - 1 and last_self))
        if pend is not None:
            for zm, za, zb in pend[4]:
                nc.vector.tensor_add(zm, za, zb)
            emit_idents(pend, 0, len(pend[0]))
            emit_join(pend)

    nc.compile()
    return nc


def _get_nc(**kw):
    key = repr(sorted(kw.items()))
    if key not in _CACHE:
        _CACHE[key] = _build(**kw)
    return _CACHE[key]


def _host_prep(inputs):
    import ml_dtypes
    x = np.asarray(inputs["input_tensor"], np.float32).reshape(NTOK_TOTAL, D)
    keys = np.asarray(inputs["memory_keys"], np.float32)
    ops = np.asarray(inputs["memory_ops"], np.float32)
    wq = np.asarray(inputs["Wq"], np.float32)
    bq = np.asarray(inputs["bq"], np.float32)

    bf16 = ml_dtypes.bfloat16
    # logits = x @ K2 + bl ; K2[d, m] = sum_e Wq[e, d] keys[m, e] / sqrt(D)
    k2 = ((keys @ wq).T * SCALE).astype(bf16)                  # [D, M]
    bl = ((keys @ bq) * SCALE).astype(np.float32)              # [M]
    bl8 = np.ascontiguousarray(np.tile(bl, GT)[None, :]).astype(bf16)
    opsT = np.ascontiguousarray(
        ops.transpose(2, 0, 1).reshape(D, M * D)).astype(bf16)  # [e, (m,d)]
    xT = [
        np.ascontiguousarray(
            np.concatenate([k2, x[c * NTOK:(c + 1) * NTOK].T.astype(bf16)],
                           axis=1))
        for c in range(N_CORES)
    ]
    return xT, opsT, k2, bl8


def _run(inputs, trace=False, **build_kw):
    nc = _get_nc(**build_kw)
    xT, opsT, k2, bl8 = _host_prep(inputs)
    in_maps = [
        {"xT": xT[c], "opsT": opsT, "k2": k2, "bl8": bl8}
        for c in range(N_CORES)
    ]
    res = run_bass_kernel_spmd(
        nc, in_maps, core_ids=list(range(N_CORES)), trace=trace
    )
    out = np.concatenate(
        [res.results[c]["out"] for c in range(N_CORES)], axis=0)
    return out.reshape(B, S, D), res


def kernel(**inputs) -> np.ndarray:
    out, _ = _run(inputs, trace=False)
    return out


# revision 7
# speedup vs baseline: 1.0262x; 1.0016x over previous
"""Trainium2 Bass kernel for CoreProcessor (soft memory-slot routing).

Computation (per token t):
    q = x Wq^T + bq
    a = softmax((q keys^T) / sqrt(d))
    out = sum_m a[m] * (ops[m] @ x)

Sharding: data-parallel over the 16384 tokens across 8 cores (2048 each);
keys/ops/Wq/bq replicated.

All static layout work is done on the HOST:
  - xT bf16 [d, 64+tok] (host-transposed, with K2 = (keys Wq)^T/sqrt(d)
    packed as a 64-col header so one DMA starts the head chain),
    opsT bf16 [e, (m,d)], bl8 bias row [1, 8m] bf16. No device prologue,
    no per-tile transposes, no ACT xT copies.
  - Heads batched 8 tiles per group: one PSUM bank of logits (8 matmuls +
    1 bias matmul), one 512-wide exp, strided Z-reduce, reciprocal, and
    1/Z folded into p up front (p8n) so no output scaling remains.
  - Mains: 16 rhs chunks of 512 per tile. PSUM: two 2-bank double
    positions + one single-bank ring (shared with the logits bank via a
    common tile tag) + two rotating pe_acc banks = 8 banks.
  - Drain routes per tile (plan "AADSADSDD" = A:3 D:4 S:2 per tile,
    grid-tuned against the cost model; Pool has no PSUM port and rejects
    TensorScalarPtr, so A goes through ACT+Pool TT):
      D: DVE 1024-wide tensor_tensor (y * p bcast) -> z bf16.
      A: ACT 1024-wide copy -> bf16 SBUF; Pool TT bcast-multiply scales.
      S: ACT per-slot scaled copies (512 chunk) -> z bf16.
    One DVE pair-merge (1024-wide bf16 add, 2x mode) halves two D-chunks'
    ident count. The m-reduction is PE identity-matmuls into pe_acc,
    interleaved between the NEXT tile's mains (z is consumed
    drain-speed-sorted: singles, DVE z, Pool z, merged z) so the in-order
    PE never stalls; the per-tile join (pe_acc -> out, DVE) is emitted
    only after every ident, then DMA out.
"""

import sys

import numpy as np

sys.path.insert(0, "/opt/trn_rl_repo")

import concourse.bass as bass  # noqa: E402
import concourse.tile as tile  # noqa: E402
from concourse import bacc, mybir  # noqa: E402
from concourse.bass_utils import run_bass_kernel_spmd  # noqa: E402
from concourse.masks import make_identity  # noqa: E402

F32 = mybir.dt.float32
BF16 = mybir.dt.bfloat16

N_CORES = 8
B, S, D, M = 4, 4096, 128, 64
NTOK_TOTAL = B * S            # 16384
NTOK = NTOK_TOTAL // N_CORES  # 2048 tokens per core
NT = NTOK // 128              # 16 token tiles per core
NCHUNK = (M * D) // 512       # 16 rhs chunks of 512 (4 slots each)
GT = 8                        # tiles per head group
SCALE = 1.0 / float(np.sqrt(np.float32(D)))

_CACHE = {}


def _build(plan=None, ids_per_main=None, merges=1, zbufs=2, ybufs=2, obufs=3, last_self=False, defer_merges=True, act_join=False, n_warm=28, merge_entry=2):
    """plan: per-parity tile route lists. Each tile's plan is a list of
    ('Dd'|'Ad', dbl_idx) doubles and ('S', sgl_idx) singles covering 16
    chunks (each Dd/Ad = 2 chunks, S = 1)."""
    from contextlib import ExitStack

    if plan is None:
        # Per-tile route tokens: D = DVE-bcast double (2 chunks),
        # A = ACT-copy + Pool-STT double, V = DVE-bcast single,
        # S = ACT slot-scaled single. Must cover 16 chunks.
        plan = {
            0: ["A", "A", "D", "S", "A", "D", "S", "D", "D"],
            1: ["A", "A", "D", "S", "A", "D", "S", "D", "D"],
        }
    # idents of the previous tile emitted after the k-th main of this tile
    if ids_per_main is None:
        ids_per_main = [6] * 5 + [2] * 7 + [1] * 4
    if isinstance(ids_per_main, int):
        ids_sched = [ids_per_main] * 24
    else:
        ids_sched = list(ids_per_main) + [0] * 24

    nc = bacc.Bacc("TRN2", target_bir_lowering=False, debug=False)

    xT_d = nc.dram_tensor("xT", [D, M + NTOK], BF16, kind="ExternalInput")
    opsT_d = nc.dram_tensor("opsT", [D, M * D], BF16, kind="ExternalInput")
    k2_d = nc.dram_tensor("k2", [D, M], BF16, kind="ExternalInput")
    bl8_d = nc.dram_tensor("bl8", [1, GT * M], BF16, kind="ExternalInput")
    out_d = nc.dram_tensor("out", [NTOK, D], F32, kind="ExternalOutput")

    with tile.TileContext(nc) as tc, ExitStack() as ctx:
        consts = ctx.enter_context(tc.tile_pool(name="consts", bufs=1))
        p_pool = ctx.enter_context(tc.tile_pool(name="p", bufs=2))
        z_pool = ctx.enter_context(tc.tile_pool(name="z", bufs=zbufs))
        y_pool = ctx.enter_context(tc.tile_pool(name="ybf", bufs=ybufs))
        acc_pool = ctx.enter_context(tc.tile_pool(name="acc", bufs=2))
        out_pool = ctx.enter_context(tc.tile_pool(name="outp", bufs=obufs))
        small = ctx.enter_context(tc.tile_pool(name="small", bufs=4))
        pa_ps = ctx.enter_context(
            tc.tile_pool(name="paps", bufs=2, space=bass.MemorySpace.PSUM)
        )
        yd_ps = ctx.enter_context(
            tc.tile_pool(name="ydps", bufs=2, space=bass.MemorySpace.PSUM)
        )
        # singles + the head's logits bank share one 2-slot ring (same tag)
        ys_ps = ctx.enter_context(
            tc.tile_pool(name="ysps", bufs=2, space=bass.MemorySpace.PSUM)
        )

        # ---- constants / inputs ----
        ident = consts.tile([128, 128], F32)
        make_identity(nc, ident)
        ident_bf = consts.tile([128, 128], BF16)
        nc.vector.tensor_copy(ident_bf, ident)
        ones_bf = consts.tile([1, 128], BF16)
        nc.vector.memset(ones_bf, 1.0)

        # PE warm-up: dummy matmuls from t~0 so the p-state ramp (2x slow
        # until 3us of continuous busy) completes before real work arrives.
        # They rotate through the pe_acc ring, which real work first needs
        # only ~1.5 tiles in.
        for w in range(n_warm):
            wrm = pa_ps.tile([128, 128], F32, tag="pacc", name="warm")
            nc.tensor.matmul(wrm, ident_bf, ident_bf, start=True, stop=True,
                             skip_group_check=True)

        # k2 rides as a 64-col header on xT: one DMA starts the head chain
        kx_sb = consts.tile([D, M + NTOK], BF16)
        k2_sb = kx_sb[:, :M]
        xT_sb = kx_sb[:, M:]
        nc.sync.dma_start(kx_sb[:, :M + 256], xT_d[:, :M + 256])
        nc.sync.dma_start(kx_sb[:, M + 256:M + NTOK // 2],
                          xT_d[:, M + 256:M + NTOK // 2])
        bl8_sb = consts.tile([1, GT * M], BF16)
        nc.gpsimd.dma_start(bl8_sb, bl8_d[:])
        # opsT quarters as separate tiles for tile-granular dep tracking;
        # q2/q3 go via the (idle) gpsimd SWDGE queue in parallel with the
        # SP HWDGE queue
        MQ = M // 4
        opsT_q = [
            consts.tile([D, MQ * D], BF16, name=f"opsT_q{q}") for q in range(4)
        ]
        nc.sync.dma_start(opsT_q[0][:, :MQ * D // 2], opsT_d[:, 0:MQ * D // 2])
        nc.sync.dma_start(opsT_q[0][:, MQ * D // 2:], opsT_d[:, MQ * D // 2:MQ * D])
        nc.sync.dma_start(xT_sb[:, NTOK // 2:], xT_d[:, M + NTOK // 2:])
        nc.sync.dma_start(opsT_q[1], opsT_d[:, MQ * D:2 * MQ * D])
        nc.sync.dma_start(opsT_q[2], opsT_d[:, 2 * MQ * D:3 * MQ * D])
        nc.sync.dma_start(opsT_q[3], opsT_d[:, 3 * MQ * D:4 * MQ * D])

        def ops_slice(lo_slot, n_slots):
            """opsT columns for slots [lo_slot, lo_slot+n_slots) (must stay
            within one quarter)."""
            q = lo_slot // MQ
            base = (lo_slot - q * MQ) * D
            return opsT_q[q][:, base:base + n_slots * D]

        # ---- heads: one group of 8 tiles ----
        heads = {}

        def emit_head(g):
            lp8 = ys_ps.tile([128, GT * M], F32, tag="ys", name="lp8")
            for k in range(GT):
                t0 = (g * GT + k) * 128
                nc.tensor.matmul(
                    lp8[:, k * M:(k + 1) * M],
                    xT_sb[:, t0:t0 + 128], k2_sb,
                    start=(k == 0), stop=False, skip_group_check=True,
                )
            nc.tensor.matmul(lp8, ones_bf, bl8_sb, start=False, stop=True,
                             skip_group_check=True)
            p8 = p_pool.tile([128, GT * M], F32, tag="p8")
            nc.scalar.activation(
                p8, lp8, mybir.ActivationFunctionType.Exp, bias=0.0, scale=1.0,
            )
            zs8 = small.tile([128, GT], F32, tag="zs")
            nc.vector.tensor_reduce(
                zs8, p8[:].rearrange("t (k m) -> t k m", m=M),
                axis=mybir.AxisListType.X, op=mybir.AluOpType.add,
            )
            rz8 = small.tile([128, GT], F32, tag="rz")
            nc.vector.reciprocal(rz8, zs8)
            p8n = p_pool.tile([128, GT * M], F32, tag="p8n")
            nc.vector.tensor_tensor(
                p8n[:].rearrange("t (k m) -> t k m", m=M),
                p8[:].rearrange("t (k m) -> t k m", m=M),
                rz8[:].unsqueeze(2).broadcast_to([128, GT, M]),
                op=mybir.AluOpType.mult,
            )
            for k in range(GT):
                heads[g * GT + k] = (p8n, k)

        def emit_idents(pend, lo, hi):
            z_tiles, pe_acc = pend[0], pend[1]
            n = len(z_tiles)
            for k in range(min(lo, n), min(hi, n)):
                nc.tensor.matmul(
                    pe_acc, ident_bf, z_tiles[k],
                    start=(k == 0), stop=(k == n - 1),
                    skip_group_check=True,
                )

        def emit_join(pend):
            """out_tile = pe_acc (+ dve_acc), then DMA."""
            _, pe_acc, dve_acc, ti = pend[:4]
            out_t = out_pool.tile([128, 128], F32)
            if dve_acc is not None:
                nc.vector.scalar_tensor_tensor(
                    out_t, pe_acc, 1.0, dve_acc,
                    op0=mybir.AluOpType.mult, op1=mybir.AluOpType.add,
                )
            elif act_join:
                nc.scalar.copy(out_t, pe_acc)
            else:
                nc.vector.tensor_copy(out_t, pe_acc)
            nc.sync.dma_start(out_d[ti * 128:(ti + 1) * 128, :], out_t)

        def emit_body(i, pend, self_drain=False):
            p8n, kk = heads.pop(i)
            pbase = kk * M
            xT_t = xT_sb[:, i * 128:(i + 1) * 128]
            pe_acc = pa_ps.tile([128, 128], F32, tag="pacc")
            pool_acc = None
            dve_acc = None
            z_tiles = []
            z_dbls = []
            routes = plan.get(i, plan[i % 2]) if isinstance(plan, dict) \
                else plan[i % 2]
            slot = 0
            nid = 0  # idents of prev emitted so far
            own_nid = 0
            ndrain = 0

            nmain = 0

            def fill_idents():
                nonlocal nid, nmain
                k = ids_sched[nmain]
                nmain += 1
                if pend:
                    emit_idents(pend, nid, nid + k)
                    nid += k

            entry_counts = []
            n_entry = 0
            for r in routes:
                n_entry += 1
                if pend and n_entry == merge_entry:
                    for zm, za, zb in pend[4]:
                        nc.vector.tensor_add(zm, za, zb)
                entry_counts.append(len(z_tiles))
                if r == "D":
                    yd = yd_ps.tile([128, 1024], F32, tag="ydd")
                    nc.tensor.matmul(yd[:, :512], xT_t, ops_slice(slot, 4),
                                     start=True, stop=True,
                                     skip_group_check=True)
                    fill_idents()
                    nc.tensor.matmul(yd[:, 512:], xT_t, ops_slice(slot + 4, 4),
                                     start=True, stop=True,
                                     skip_group_check=True)
                    fill_idents()
                    zd = z_pool.tile([128, 1024], BF16, tag=f"zd{slot}",
                                     name=f"zd{slot}")
                    nc.vector.tensor_tensor(
                        zd[:].rearrange("t (m e) -> t m e", e=128),
                        yd[:].rearrange("t (m e) -> t m e", e=128),
                        p8n[:, pbase + slot:pbase + slot + 8]
                            .unsqueeze(2).broadcast_to([128, 8, 128]),
                        op=mybir.AluOpType.mult,
                    )
                    if self_drain:
                        for j in range(8):
                            z_tiles.append(zd[:, j * 128:(j + 1) * 128])
                    else:
                        z_dbls.append((0, zd))
                    slot += 8
                elif r == "P":
                    # A-pair: two doubles copied into one 2048-wide bf16
                    # buffer, ONE Pool TT scales all 16 slots
                    yb2 = y_pool.tile([128, 2048], BF16, tag=f"yb2{slot}",
                                      name=f"yb2{slot}")
                    for h in range(2):
                        yd = yd_ps.tile([128, 1024], F32, tag="ydd")
                        nc.tensor.matmul(yd[:, :512], xT_t,
                                         ops_slice(slot + 8 * h, 4),
                                         start=True, stop=True,
                                         skip_group_check=True)
                        fill_idents()
                        nc.tensor.matmul(yd[:, 512:], xT_t,
                                         ops_slice(slot + 8 * h + 4, 4),
                                         start=True, stop=True,
                                         skip_group_check=True)
                        fill_idents()
                        nc.scalar.copy(yb2[:, h * 1024:(h + 1) * 1024], yd)
                    za2 = z_pool.tile([128, 2048], BF16, tag=f"za2{slot}",
                                      name=f"za2{slot}")
                    nc.gpsimd.tensor_tensor(
                        za2[:].rearrange("t (m e) -> t m e", e=128),
                        yb2[:].rearrange("t (m e) -> t m e", e=128),
                        p8n[:, pbase + slot:pbase + slot + 16]
                            .unsqueeze(2).broadcast_to([128, 16, 128]),
                        op=mybir.AluOpType.mult,
                    )
                    z_dbls.append((1, za2[:, :1024]))
                    z_dbls.append((1, za2[:, 1024:]))
                    slot += 16
                elif r == "B":
                    # like A but the bcast scale runs on DVE (SBUF, 1x)
                    yd = yd_ps.tile([128, 1024], F32, tag="ydd")
                    nc.tensor.matmul(yd[:, :512], xT_t, ops_slice(slot, 4),
                                     start=True, stop=True,
                                     skip_group_check=True)
                    fill_idents()
                    nc.tensor.matmul(yd[:, 512:], xT_t, ops_slice(slot + 4, 4),
                                     start=True, stop=True,
                                     skip_group_check=True)
                    fill_idents()
                    yb = y_pool.tile([128, 1024], BF16, tag=f"yb{slot}",
                                     name=f"yb{slot}")
                    nc.scalar.copy(yb, yd)
                    za = z_pool.tile([128, 1024], BF16, tag=f"za{slot}",
                                     name=f"za{slot}")
                    nc.vector.tensor_tensor(
                        za[:].rearrange("t (m e) -> t m e", e=128),
                        yb[:].rearrange("t (m e) -> t m e", e=128),
                        p8n[:, pbase + slot:pbase + slot + 8]
                            .unsqueeze(2).broadcast_to([128, 8, 128]),
                        op=mybir.AluOpType.mult,
                    )
                    z_dbls.append((1, za))
                    slot += 8
                elif r == "A":
                    yd = yd_ps.tile([128, 1024], F32, tag="ydd")
                    nc.tensor.matmul(yd[:, :512], xT_t, ops_slice(slot, 4),
                                     start=True, stop=True,
                                     skip_group_check=True)
                    fill_idents()
                    nc.tensor.matmul(yd[:, 512:], xT_t, ops_slice(slot + 4, 4),
                                     start=True, stop=True,
                                     skip_group_check=True)
                    fill_idents()
                    yb = y_pool.tile([128, 1024], BF16, tag=f"yb{slot}",
                                     name=f"yb{slot}")
                    nc.scalar.copy(yb, yd)
                    za = z_pool.tile([128, 1024], BF16, tag=f"za{slot}",
                                     name=f"za{slot}")
                    nc.gpsimd.tensor_tensor(
                        za[:].rearrange("t (m e) -> t m e", e=128),
                        yb[:].rearrange("t (m e) -> t m e", e=128),
                        p8n[:, pbase + slot:pbase + slot + 8]
                            .unsqueeze(2).broadcast_to([128, 8, 128]),
                        op=mybir.AluOpType.mult,
                    )
                    if self_drain:
                        for j in range(8):
                            z_tiles.append(za[:, j * 128:(j + 1) * 128])
                    else:
                        z_dbls.append((1, za))
                    slot += 8
                elif r == "V":
                    ys = ys_ps.tile([128, 512], F32, tag="ys", name="ys")
                    nc.tensor.matmul(ys, xT_t, ops_slice(slot, 4),
                                     start=True, stop=True,
                                     skip_group_check=True)
                    fill_idents()
                    zv = z_pool.tile([128, 512], BF16, tag=f"zv{slot}",
                                     name=f"zv{slot}")
                    nc.vector.tensor_tensor(
                        zv[:].rearrange("t (m e) -> t m e", e=128),
                        ys[:].rearrange("t (m e) -> t m e", e=128),
                        p8n[:, pbase + slot:pbase + slot + 4]
                            .unsqueeze(2).broadcast_to([128, 4, 128]),
                        op=mybir.AluOpType.mult,
                    )
                    for j in range(4):
                        z_tiles.append(zv[:, j * 128:(j + 1) * 128])
                    slot += 4
                elif r == "N":
                    ys = ys_ps.tile([128, 512], F32, tag="ys", name="ys")
                    nc.tensor.matmul(ys, xT_t, ops_slice(slot, 4),
                                     start=True, stop=True,
                                     skip_group_check=True)
                    fill_idents()
                    for j in range(4):
                        col = p8n[:, pbase + slot + j:pbase + slot + j + 1]
                        if dve_acc is None:
                            dve_acc = acc_pool.tile([128, 128], F32,
                                                    tag="dveacc")
                            nc.vector.tensor_scalar_mul(
                                dve_acc, ys[:, j * 128:(j + 1) * 128], col)
                        else:
                            nc.vector.scalar_tensor_tensor(
                                dve_acc, ys[:, j * 128:(j + 1) * 128], col,
                                dve_acc, op0=mybir.AluOpType.mult,
                                op1=mybir.AluOpType.add)
                    slot += 4
                else:  # "S"
                    ys = ys_ps.tile([128, 512], F32, tag="ys", name="ys")
                    nc.tensor.matmul(ys, xT_t, ops_slice(slot, 4),
                                     start=True, stop=True,
                                     skip_group_check=True)
                    fill_idents()
                    for j in range(4):
                        zs = z_pool.tile([128, 128], BF16, tag=f"zs{slot + j}",
                                         name=f"zs{slot + j}")
                        nc.scalar.mul(
                            zs, ys[:, j * 128:(j + 1) * 128],
                            p8n[:, pbase + slot + j:pbase + slot + j + 1],
                        )
                        z_tiles.append(zs)
                    slot += 4
                ndrain += 2 if r in ("D", "A") else 1
                lag_n = (entry_counts[-3] if len(entry_counts) >= 3
                         else 0)
                if self_drain and lag_n > own_nid:
                    for k in range(own_nid, lag_n):
                        nc.tensor.matmul(
                            pe_acc, ident_bf, z_tiles[k],
                            start=(k == 0), stop=False,
                            skip_group_check=True,
                        )
                    own_nid = lag_n
            assert slot == M, f"plan covers {slot} slots"
            # sort by drain-engine speed: DVE z first, Pool-scaled z last
            z_dbls.sort(key=lambda kz: kz[0])
            # pair-merge the FIRST z doubles on DVE (bf16 2x adds) to cut
            # their ident count; the adds are DEFERRED into the next
            # tile's stream (emitted via closures) so they don't delay
            # this tile's ring-critical drains
            merge_ops = []
            ds = [z for kk, z in z_dbls if kk == 0]
            as_ = [z for kk, z in z_dbls if kk == 1]
            zms = []
            # pair selection: latest-ready pairs first (A-pair, then late
            # D-pairs) so early-ready z stays at the consumption front
            cand = []
            if len(ds) >= 2:
                cand.append((ds[-2], ds[-1], "D"))
            if len(as_) >= 2:
                cand.append((as_[-2], as_[-1], "A"))
            if len(ds) >= 4:
                cand.append((ds[-4], ds[-3], "D2"))
            used = set()
            for k in range(min(merges, len(cand))):
                za, zb, _ = cand[k]
                used.add(id(za)); used.add(id(zb))
                zm = z_pool.tile([128, 1024], BF16, tag=f"zm{k}",
                                 name=f"zm{k}")
                merge_ops.append((zm, za, zb))
                zms.append(zm)
            z_dbls = ([(0, z) for z in ds if id(z) not in used]
                      + [(1, z) for z in as_ if id(z) not in used]
                      + [(2, z) for z in zms])
            if not defer_merges:
                for zm, za, zb in merge_ops:
                    nc.vector.tensor_add(zm, za, zb)
                merge_ops = []
            for _, zd in z_dbls:
                for j in range(8):
                    z_tiles.append(zd[:, j * 128:(j + 1) * 128])
            if pend:
                emit_idents(pend, nid, len(pend[0]))  # flush stragglers
                emit_join(pend)
            if self_drain:
                n = len(z_tiles)
                for k in range(own_nid, n):
                    nc.tensor.matmul(
                        pe_acc, ident_bf, z_tiles[k],
                        start=(k == 0), stop=(k == n - 1),
                        skip_group_check=True,
                    )
                if own_nid >= n:
                    # all emitted with stop=False; emit a closing no-op
                    # accumulation of the last z to terminate the group
                    nc.tensor.matmul(
                        pe_acc, ident_bf, z_tiles[n - 1],
                        start=False, stop=True, skip_group_check=True,
                    )
                    raise AssertionError("unreachable: lag ensures tail")
                emit_join((None, pe_acc, dve_acc, i, []))
                return None
            return (z_tiles, pe_acc, dve_acc, i, merge_ops)

        emit_head(0)
        pend = None
        for i in range(NT):
            if i % GT == GT - 3 and i // GT + 1 < NT // GT:
                emit_head(i // GT + 1)
            pend = emit_body(i, pend, self_drain=(i == NT - 1 and last_self))
        if pend is not None:
            for zm, za, zb in pend[4]:
                nc.vector.tensor_add(zm, za, zb)
            emit_idents(pend, 0, len(pend[0]))
            emit_join(pend)

    nc.compile()
    return nc


def _get_nc(**kw):
    key = repr(sorted(kw.items()))
    if key not in _CACHE:
        _CACHE[key] = _build(**kw)
    return _CACHE[key]


def _host_prep(inputs):
    import ml_dtypes
    x = np.asarray(inputs["input_tensor"], np.float32).reshape(NTOK_TOTAL, D)
    keys = np.asarray(inputs["memory_keys"], np.float32)
    ops = np.asarray(inputs["memory_ops"], np.float32)
    wq = np.asarray(inputs["Wq"], np.float32)
    bq = np.asarray(inputs["bq"], np.float32)

    bf16 = ml_dtypes.bfloat16
    # logits = x @ K2 + bl ; K2[d, m] = sum_e Wq[e, d] keys[m, e] / sqrt(D)
    k2 = ((keys @ wq).T * SCALE).astype(bf16)                  # [D, M]
    bl = ((keys @ bq) * SCALE).astype(np.float32)              # [M]
    bl8 = np.ascontiguousarray(np.tile(bl, GT)[None, :]).astype(bf16)
    opsT = np.ascontiguousarray(
        ops.transpose(2, 0, 1).reshape(D, M * D)).astype(bf16)  # [e, (m,d)]
    xT = [
        np.ascontiguousarray(
            np.concatenate([k2, x[c * NTOK:(c + 1) * NTOK].T.astype(bf16)],
                           axis=1))
        for c in range(N_CORES)
    ]
    return xT, opsT, k2, bl8


def _run(inputs, trace=False, **build_kw):
    nc = _get_nc(**build_kw)
    xT, opsT, k2, bl8 = _host_prep(inputs)
    in_maps = [
        {"xT": xT[c], "opsT": opsT, "k2": k2, "bl8": bl8}
        for c in range(N_CORES)
    ]
    res = run_bass_kernel_spmd(
        nc, in_maps, core_ids=list(range(N_CORES)), trace=trace
    )
    out = np.concatenate(
        [res.results[c]["out"] for c in range(N_CORES)], axis=0)
    return out.reshape(B, S, D), res


def kernel(**inputs) -> np.ndarray:
    out, _ = _run(inputs, trace=False)
    return out
